# revision 2
# baseline (speedup 1.0000x reference)
"""Trainium2 Bass kernel for nn_CitationClassifier (pooling/ridge).

Strategy: pure data parallel over the batch dim (256 = 8 cores x 32).
Each core:
  - token scan (find '@' span, CITSEG pos) on DVE in f32 math
  - span-masked max-pool over S via mask-add (ACT+DVE) + pairwise max
    (DVE) + PE-transpose + free-dim max reduce -> pooled^T [768, 32]
  - CITSEG row gather via indirect DMA, projection + encoder GEMMs
    (feature-major, weights stationary)
  - 3-layer MLP (batch-major: activations stationary, weights moving)
  - all matmuls run as float32r (full-rate fp32 streaming)
Output [32, 6] per core, concatenated on host to [256, 6].
"""

import sys

for _p in ("/opt/trn_rl_repo", "/root/.axon_site/_ro/trn_rl_repo"):
    if _p not in sys.path:
        sys.path.insert(0, _p)

import numpy as np

# --- problem dims (hardcoded per harness contract) ---
B, S, H = 256, 512, 768
CIT, D1, D2, NCLS = 750, 1518, 3036, 6
NCORES = 8
BPC = B // NCORES  # 32 samples per core
P = 128
AT_ID, CITSEG_ID = 5, 7
BIGF = 3.4028e38  # large negative fill (finite, f32-safe)

_CACHED = {}


def _build_bass():
    import os
    SKIP = set(os.environ.get("KB_SKIP", "").split(","))
    from concourse import bacc, bass, mybir
    import concourse.tile as tile
    from concourse.masks import make_identity

    dt = mybir.dt
    op = mybir.AluOpType
    act = mybir.ActivationFunctionType
    ax = mybir.AxisListType

    f32, i32 = dt.float32, dt.int32
    # float32r needs producers to emit FP32R-rounded outputs (BIR verifier);
    # plain f32 keeps the graph simple — PE is not the bottleneck in the
    # data-parallel version.
    f32r = f32

    nc = bacc.Bacc("TRN2", target_bir_lowering=False, debug=False)

    # ---- DRAM parameters (per-core shard shapes) ----
    tokens_d = nc.declare_dram_parameter("tokens", [BPC, S], i32, isOutput=False)
    hidden_d = nc.declare_dram_parameter("hidden", [BPC, S, H], f32, isOutput=False)
    projw_d = nc.declare_dram_parameter("proj_w", [H, CIT], f32, isOutput=False)
    projb_d = nc.declare_dram_parameter("proj_b", [1, CIT], f32, isOutput=False)
    encw_d = nc.declare_dram_parameter("enc_w", [CIT, CIT], f32, isOutput=False)
    encb_d = nc.declare_dram_parameter("enc_b", [1, CIT], f32, isOutput=False)
    w1_d = nc.declare_dram_parameter("w1", [D1, D2], f32, isOutput=False)
    b1_d = nc.declare_dram_parameter("b1", [1, D2], f32, isOutput=False)
    w2_d = nc.declare_dram_parameter("w2", [D2, D2], f32, isOutput=False)
    b2_d = nc.declare_dram_parameter("b2", [1, D2], f32, isOutput=False)
    w3_d = nc.declare_dram_parameter("w3", [D2, NCLS], f32, isOutput=False)
    b3_d = nc.declare_dram_parameter("b3", [1, NCLS], f32, isOutput=False)
    out_d = nc.declare_dram_parameter("out", [BPC, NCLS], f32, isOutput=True)

    # k-tiling helpers
    def ktiles(total):
        ts = []
        off = 0
        while off < total:
            sz = min(P, total - off)
            ts.append((off, sz))
            off += sz
        return ts

    KT_X = ktiles(D1)   # 12 tiles: 11x128 + 110
    KT_H = ktiles(D2)   # 24 tiles: 23x128 + 92
    KT_HID = ktiles(H)  # 6x128
    KT_CIT = ktiles(CIT)  # 5x128 + 110
    NG = 3
    GW = D2 // NG  # 1012 per psum group (2 PSUM banks)
    # within-group matmul col chunks (PSUM bank = 512 f32)
    CHUNKS = [(0, 512), (512, GW - 512)]

    with tile.TileContext(nc) as tc:
        with (
            tc.tile_pool(name="consts", bufs=1) as cpool,
            tc.tile_pool(name="hb", bufs=3) as hbp,
            tc.tile_pool(name="mx", bufs=2) as mxp,
            tc.tile_pool(name="wmov", bufs=4) as wp,
            tc.tile_pool(name="psptr", bufs=2, space="PSUM") as psp,
            tc.tile_pool(name="psgemm", bufs=2, space="PSUM") as psb,
            tc.tile_pool(name="pssmall", bufs=2, space="PSUM") as pss,
        ):
            # ---------- constants ----------
            ident = cpool.tile([P, P], f32, tag="ident")
            make_identity(nc, ident[:])
            ones_col = cpool.tile([1, BPC], f32, tag="ones_col")
            nc.vector.memset(ones_col[:], 1.0)
            ones_row = cpool.tile([1, P], f32, tag="ones_row")
            nc.vector.memset(ones_row[:], 1.0)

            # ---------- phase 0: token scan ----------
            tok_i = cpool.tile([BPC, S], i32, tag="tok_i")
            nc.sync.dma_start(out=tok_i[:], in_=tokens_d[:])
            tok = cpool.tile([BPC, S], f32, tag="tok")
            nc.vector.tensor_copy(out=tok[:], in_=tok_i[:])

            iota_i = cpool.tile([BPC, S], i32, tag="iota_i")
            nc.gpsimd.iota(iota_i[:], pattern=[[1, S]], base=0, channel_multiplier=0)
            iot = cpool.tile([BPC, S], f32, tag="iot")
            nc.vector.tensor_copy(out=iot[:], in_=iota_i[:])

            biota_i = cpool.tile([BPC, 1], i32, tag="biota_i")
            nc.gpsimd.iota(biota_i[:], pattern=[[0, 1]], base=0, channel_multiplier=1)
            biota = cpool.tile([BPC, 1], f32, tag="biota")
            nc.vector.tensor_copy(out=biota[:], in_=biota_i[:])

            def ts_(out_ap, in_ap, s1, o1, s2=None, o2=op.bypass):
                nc.vector.tensor_scalar(
                    out=out_ap, in0=in_ap, scalar1=s1, scalar2=s2, op0=o1, op1=o2
                )

            def tt_(out_ap, a, b_, o):
                nc.vector.tensor_tensor(out=out_ap, in0=a, in1=b_, op=o)

            def bc(ap_, shape):
                return ap_.to_broadcast(shape)

            t_a = cpool.tile([BPC, S], f32, tag="t_a")   # scratch a
            t_b = cpool.tile([BPC, S], f32, tag="t_b")   # scratch b
            t_c = cpool.tile([BPC, S], f32, tag="t_c")   # scratch c (holds 'at')
            s_1 = cpool.tile([BPC, 1], f32, tag="s_1")
            first = cpool.tile([BPC, 1], f32, tag="first")
            second = cpool.tile([BPC, 1], f32, tag="second")
            ge2 = cpool.tile([BPC, 1], f32, tag="ge2")
            start = cpool.tile([BPC, 1], f32, tag="start")
            end = cpool.tile([BPC, 1], f32, tag="end")
            keep = cpool.tile([BPC, S], f32, tag="keep")
            keepany = cpool.tile([BPC, 1], f32, tag="keepany")
            maskneg = cpool.tile([BPC, S], f32, tag="maskneg")
            hasc = cpool.tile([BPC, 1], f32, tag="hasc")
            spos = cpool.tile([BPC, 1], f32, tag="spos")
            gidx_f = cpool.tile([BPC, 1], f32, tag="gidx_f")
            gidx_i = cpool.tile([BPC, 1], i32, tag="gidx_i")

            # at = (tok == 5); penalty = at*-1000+1000; t = penalty+iota
            ts_(t_c[:], tok[:], float(AT_ID), op.is_equal)
            ts_(t_a[:], t_c[:], -1000.0, op.mult, 1000.0, op.add)
            tt_(t_a[:], t_a[:], iot[:], op.add)
            nc.vector.tensor_reduce(out=first[:], in_=t_a[:], axis=ax.X, op=op.min)
            # second: at & (iota > first)
            tt_(t_b[:], iot[:], bc(first[:], [BPC, S]), op.is_gt)
            tt_(t_b[:], t_b[:], t_c[:], op.mult)
            ts_(t_b[:], t_b[:], -1000.0, op.mult, 1000.0, op.add)
            tt_(t_b[:], t_b[:], iot[:], op.add)
            nc.vector.tensor_reduce(out=second[:], in_=t_b[:], axis=ax.X, op=op.min)
            # ge2 = (sum(at) >= 2)
            nc.vector.tensor_reduce(out=s_1[:], in_=t_c[:], axis=ax.X, op=op.add)
            ts_(ge2[:], s_1[:], 2.0, op.is_ge)
            # start = first*ge2 ; end = (second-512)*ge2 + 512
            tt_(start[:], first[:], ge2[:], op.mult)
            ts_(end[:], second[:], -float(S), op.add)
            tt_(end[:], end[:], ge2[:], op.mult)
            ts_(end[:], end[:], float(S), op.add)
            # keep = (iota < start) | (iota > end)
            tt_(t_a[:], iot[:], bc(start[:], [BPC, S]), op.is_lt)
            tt_(t_b[:], iot[:], bc(end[:], [BPC, S]), op.is_gt)
            tt_(keep[:], t_a[:], t_b[:], op.max)
            nc.vector.tensor_reduce(out=keepany[:], in_=keep[:], axis=ax.X, op=op.max)
            # maskneg = (keep-1)*BIGF  -> 0 where keep, -BIGF where masked
            ts_(maskneg[:], keep[:], -1.0, op.add, BIGF, op.mult)
            # CITSEG first occurrence
            ts_(t_c[:], tok[:], float(CITSEG_ID), op.is_equal)
            ts_(t_a[:], t_c[:], -1000.0, op.mult, 1000.0, op.add)
            tt_(t_a[:], t_a[:], iot[:], op.add)
            nc.vector.tensor_reduce(out=s_1[:], in_=t_a[:], axis=ax.X, op=op.min)
            ts_(hasc[:], s_1[:], float(S - 1), op.is_le)
            ts_(spos[:], s_1[:], float(S - 1), op.min)
            # gather row index = b*512 + spos
            ts_(gidx_f[:], biota[:], float(S), op.mult)
            tt_(gidx_f[:], gidx_f[:], spos[:], op.add)
            nc.vector.tensor_copy(out=gidx_i[:], in_=gidx_f[:])

            # ---------- transpose masks to [128 s, chunk, 32 b] ----------
            maskcols = cpool.tile([P, 4, BPC], f32, tag="maskcols")
            if "mask" in SKIP:
                nc.vector.memset(maskcols[:], 0.0)
            for c in range(4 if "mask" not in SKIP else 0):
                pt = pss.tile([P, BPC], f32, tag="pt_small")
                nc.tensor.transpose(
                    out=pt[:], in_=maskneg[:, c * P:(c + 1) * P], identity=ident[:BPC, :BPC]
                )
                nc.vector.tensor_copy(out=maskcols[:, c, :], in_=pt[:])

            # hasc_row [1, 32], keepany broadcast [128, 32]
            hasc_row = cpool.tile([1, BPC], f32, tag="hasc_row")
            pt = pss.tile([P, BPC], f32, tag="pt_small")
            nc.tensor.transpose(out=pt[:1, :], in_=hasc[:], identity=ident[:BPC, :BPC])
            nc.vector.tensor_copy(out=hasc_row[:], in_=pt[:1, :])

            ka_row = cpool.tile([1, BPC], f32, tag="ka_row")
            pt = pss.tile([P, BPC], f32, tag="pt_small")
            nc.tensor.transpose(out=pt[:1, :], in_=keepany[:], identity=ident[:BPC, :BPC])
            nc.vector.tensor_copy(out=ka_row[:], in_=pt[:1, :])
            kab = cpool.tile([P, BPC], f32, tag="kab")
            pt = pss.tile([P, BPC], f32, tag="pt_small")
            nc.tensor.matmul(out=pt[:], lhsT=ones_row[:], rhs=ka_row[:], start=True, stop=True)
            nc.vector.tensor_copy(out=kab[:], in_=pt[:])

            # ---------- CITSEG gather + cit_h^T ----------
            cith = cpool.tile([BPC, H], f32, tag="cith")
            hid_flat = hidden_d[:].rearrange("b s h -> (b s) h")
            if "cit" in SKIP:
                nc.vector.memset(cith[:], 0.001)
            else:
                nc.gpsimd.indirect_dma_start(
                    out=cith[:],
                    out_offset=None,
                    in_=hid_flat,
                    in_offset=bass.IndirectOffsetOnAxis(ap=gidx_i[:, :1], axis=0),
                )
            tt_(cith[:], cith[:], bc(hasc[:], [BPC, H]), op.mult)
            cithT = cpool.tile([P, len(KT_HID), BPC], f32, tag="cithT")
            for t in range(len(KT_HID)):
                pt = pss.tile([P, BPC], f32, tag="pt_small")
                nc.tensor.transpose(
                    out=pt[:], in_=cith[:, t * P:(t + 1) * P], identity=ident[:BPC, :BPC]
                )
                nc.vector.tensor_copy(out=cithT[:, t, :], in_=pt[:])

            # ---------- pooling over S (per sample) ----------
            xT = cpool.tile([P, len(KT_X), BPC], f32, tag="xT")
            if "pool" in SKIP:
                nc.vector.memset(xT[:], 0.001)
            for b in range(BPC if "pool" not in SKIP else 0):
                hb = hbp.tile([P, 4, H], f32, tag="hb")
                nc.sync.dma_start(
                    out=hb[:], in_=hidden_d[b].rearrange("(c p) h -> p c h", p=P)
                )
                # masked add (in place): 3 chunks on ACT, 1 on DVE
                for c in range(4):
                    bias_ap = maskcols[:, c, b:b + 1]
                    if c < 3:
                        nc.scalar.activation(
                            out=hb[:, c, :], in_=hb[:, c, :],
                            func=act.Identity, bias=bias_ap, scale=1.0,
                        )
                    else:
                        nc.vector.tensor_scalar(
                            out=hb[:, c, :], in0=hb[:, c, :],
                            scalar1=bias_ap, scalar2=None, op0=op.add,
                        )
                mx = mxp.tile([P, 2, H], f32, tag="mx")
                tt_(mx[:], hb[:, 0:2, :], hb[:, 2:4, :], op.max)
                acc = mxp.tile([P, H], f32, tag="acc")
                tt_(acc[:], mx[:, 0, :], mx[:, 1, :], op.max)
                # transpose h-chunks to psum (1-bank halves); reduce over s-lanes
                for half in range(2):
                    ptr = psp.tile([P, 3, P], f32, tag="ptr")
                    for t in range(3):
                        hc = half * 3 + t
                        nc.tensor.transpose(
                            out=ptr[:, t, :], in_=acc[:, hc * P:(hc + 1) * P],
                            identity=ident[:],
                        )
                    nc.vector.tensor_reduce(
                        out=xT[:, half * 3:half * 3 + 3, b], in_=ptr[:],
                        axis=ax.X, op=op.max,
                    )
            # zero pooled where no kept position
            for t in range(6):
                nc.vector.tensor_tensor(
                    out=xT[:, t, :], in0=xT[:, t, :], in1=kab[:], op=op.mult
                )

            # ---------- proj (feature-major) ----------
            if "projenc" in SKIP:
                nc.vector.memset(xT[:, 6:12, :], 0.001)
            projw_sb = cpool.tile([P, len(KT_HID), CIT], f32, tag="projw_sb")
            if "projenc" in SKIP:
                KT_CIT_EFF = []
            else:
                KT_CIT_EFF = KT_CIT
            nc.sync.dma_start(out=projw_sb[:], in_=projw_d[:].rearrange("(t p) m -> p t m", p=P))
            projb_sb = cpool.tile([1, CIT], f32, tag="projb_sb")
            nc.sync.dma_start(out=projb_sb[:], in_=projb_d[:])
            cpT = cpool.tile([P, len(KT_CIT), BPC], f32, tag="cpT")
            for mt, (moff, msz) in enumerate(KT_CIT_EFF):
                ps = pss.tile([P, BPC], f32, tag="pt_small")
                for kt in range(len(KT_HID)):
                    nc.tensor.matmul(
                        out=ps[:msz, :],
                        lhsT=projw_sb[:, kt, moff:moff + msz].bitcast(f32r),
                        rhs=cithT[:, kt, :].bitcast(f32r),
                        start=(kt == 0), stop=False,
                    )
                nc.tensor.matmul(
                    out=ps[:msz, :], lhsT=projb_sb[:, moff:moff + msz].bitcast(f32r),
                    rhs=hasc_row[:].bitcast(f32r), start=False, stop=True,
                )
                nc.vector.tensor_copy(out=cpT[:msz, mt, :], in_=ps[:msz, :])

            # ---------- encoder (feature-major) ----------
            encw_sb = cpool.tile([P, len(KT_CIT), CIT], f32, tag="encw_sb")
            nc.vector.memset(encw_sb[:], 0.0)
            nfull = CIT // P  # 5
            nc.sync.dma_start(
                out=encw_sb[:, 0:nfull, :],
                in_=encw_d[0:nfull * P, :].rearrange("(t p) m -> p t m", p=P),
            )
            nc.sync.dma_start(out=encw_sb[0:CIT - nfull * P, nfull, :], in_=encw_d[nfull * P:CIT, :])
            encb_sb = cpool.tile([1, CIT], f32, tag="encb_sb")
            nc.sync.dma_start(out=encb_sb[:], in_=encb_d[:])
            for mt, (moff, msz) in enumerate(KT_CIT_EFF):
                ps = pss.tile([P, BPC], f32, tag="pt_small")
                for kt, (koff, ksz) in enumerate(KT_CIT):
                    nc.tensor.matmul(
                        out=ps[:msz, :],
                        lhsT=encw_sb[:ksz, kt, moff:moff + msz].bitcast(f32r),
                        rhs=cpT[:ksz, kt, :].bitcast(f32r),
                        start=(kt == 0), stop=False,
                    )
                nc.tensor.matmul(
                    out=ps[:msz, :], lhsT=encb_sb[:, moff:moff + msz].bitcast(f32r),
                    rhs=ones_col[:].bitcast(f32r), start=False, stop=True,
                )
                nc.vector.tensor_copy(out=xT[:msz, 6 + mt, :], in_=ps[:msz, :])

            # ---------- MLP (batch-major: x stationary, W moving) ----------
            def mlp_layer(x_kt_tiles, kt_list, w_dram, b_sb, h_out, n_out, relu):
                if "mlp" in SKIP:
                    nc.vector.memset(h_out[:], 0.001)
                    return
                for g in range(NG if n_out == D2 else 1):
                    goff = g * GW
                    gw = GW if n_out == D2 else n_out
                    psg = psb.tile([BPC, GW], f32, tag="psgemm")
                    chunks = CHUNKS if n_out == D2 else [(0, n_out)]
                    for kt, (koff, ksz) in enumerate(kt_list):
                        wt = wp.tile([P, GW], f32, tag="wt")
                        nc.sync.dma_start(
                            out=wt[:ksz, :gw], in_=w_dram[koff:koff + ksz, goff:goff + gw]
                        )
                        for (c0, cw) in chunks:
                            nc.tensor.matmul(
                                out=psg[:, c0:c0 + cw],
                                lhsT=x_kt_tiles(kt, ksz).bitcast(f32r),
                                rhs=wt[:ksz, c0:c0 + cw].bitcast(f32r),
                                start=(kt == 0), stop=False,
                            )
                    for ci, (c0, cw) in enumerate(chunks):
                        nc.tensor.matmul(
                            out=psg[:, c0:c0 + cw],
                            lhsT=ones_col[:].bitcast(f32r),
                            rhs=b_sb[:, goff + c0:goff + c0 + cw].bitcast(f32r),
                            start=False, stop=True,
                        )
                    if relu:
                        nc.scalar.activation(
                            out=h_out[:, goff:goff + gw], in_=psg[:, :gw], func=act.Relu
                        )
                    else:
                        nc.vector.tensor_copy(out=h_out[:, goff:goff + gw], in_=psg[:, :gw])

            b1_sb = cpool.tile([1, D2], f32, tag="b1_sb")
            nc.sync.dma_start(out=b1_sb[:], in_=b1_d[:])
            b2_sb = cpool.tile([1, D2], f32, tag="b2_sb")
            nc.sync.dma_start(out=b2_sb[:], in_=b2_d[:])
            b3_sb = cpool.tile([1, NCLS], f32, tag="b3_sb")
            nc.sync.dma_start(out=b3_sb[:], in_=b3_d[:])

            h1 = cpool.tile([BPC, D2], f32, tag="h1")
            mlp_layer(lambda kt, ksz: xT[:ksz, kt, :], KT_X, w1_d, b1_sb, h1, D2, True)

            # transpose h1 -> h1T k-tiles
            h1T = cpool.tile([P, len(KT_H), BPC], f32, tag="h1T")
            KT_H_EFF = KT_H if "mlp" not in SKIP else []
            for t, (toff, tsz) in enumerate(KT_H_EFF):
                pt = pss.tile([P, BPC], f32, tag="pt_small")
                nc.tensor.transpose(
                    out=pt[:tsz, :], in_=h1[:, toff:toff + tsz], identity=ident[:BPC, :BPC]
                )
                nc.vector.tensor_copy(out=h1T[:tsz, t, :], in_=pt[:tsz, :])

            h2 = cpool.tile([BPC, D2], f32, tag="h2")
            mlp_layer(lambda kt, ksz: h1T[:ksz, kt, :], KT_H, w2_d, b2_sb, h2, D2, True)

            h2T = cpool.tile([P, len(KT_H), BPC], f32, tag="h2T")
            for t, (toff, tsz) in enumerate(KT_H_EFF):
                pt = pss.tile([P, BPC], f32, tag="pt_small")
                nc.tensor.transpose(
                    out=pt[:tsz, :], in_=h2[:, toff:toff + tsz], identity=ident[:BPC, :BPC]
                )
                nc.vector.tensor_copy(out=h2T[:tsz, t, :], in_=pt[:tsz, :])

            # L3: w3 resident
            w3_sb = cpool.tile([P, len(KT_H), NCLS], f32, tag="w3_sb")
            nc.vector.memset(w3_sb[:], 0.0)
            nfull3 = D2 // P  # 23
            nc.sync.dma_start(
                out=w3_sb[:, 0:nfull3, :],
                in_=w3_d[0:nfull3 * P, :].rearrange("(t p) m -> p t m", p=P),
            )
            nc.sync.dma_start(out=w3_sb[0:D2 - nfull3 * P, nfull3, :], in_=w3_d[nfull3 * P:D2, :])

            pso = pss.tile([P, BPC], f32, tag="pt_small")
            if "mlp" in SKIP:
                nc.vector.memset(h2T[:], 0.001)
            for kt, (koff, ksz) in enumerate(KT_H):
                nc.tensor.matmul(
                    out=pso[:BPC, :NCLS],
                    lhsT=h2T[:ksz, kt, :].bitcast(f32r),
                    rhs=w3_sb[:ksz, kt, :].bitcast(f32r),
                    start=(kt == 0), stop=False,
                )
            nc.tensor.matmul(
                out=pso[:BPC, :NCLS], lhsT=ones_col[:].bitcast(f32r),
                rhs=b3_sb[:].bitcast(f32r), start=False, stop=True,
            )
            out_sb = cpool.tile([BPC, NCLS], f32, tag="out_sb")
            nc.vector.tensor_copy(out=out_sb[:], in_=pso[:BPC, :NCLS])
            nc.sync.dma_start(out=out_d[:], in_=out_sb[:])

    nc.compile()
    return nc


def _get_nc():
    if "nc" not in _CACHED:
        _CACHED["nc"] = _build_bass()
    return _CACHED["nc"]


def kernel(**inputs) -> np.ndarray:
    from concourse.bass_utils import run_bass_kernel_spmd

    nc = _get_nc()

    tokens = np.asarray(inputs["tokens"]).astype(np.int32)
    hidden = np.ascontiguousarray(np.asarray(inputs["hidden_states"], dtype=np.float32))
    shared = {
        "proj_w": np.ascontiguousarray(inputs["proj_w"], dtype=np.float32),
        "proj_b": np.asarray(inputs["proj_b"], dtype=np.float32).reshape(1, CIT),
        "enc_w": np.ascontiguousarray(inputs["enc_w"], dtype=np.float32),
        "enc_b": np.asarray(inputs["enc_b"], dtype=np.float32).reshape(1, CIT),
        "w1": np.ascontiguousarray(inputs["w1"], dtype=np.float32),
        "b1": np.asarray(inputs["b1"], dtype=np.float32).reshape(1, D2),
        "w2": np.ascontiguousarray(inputs["w2"], dtype=np.float32),
        "b2": np.asarray(inputs["b2"], dtype=np.float32).reshape(1, D2),
        "w3": np.ascontiguousarray(inputs["w3"], dtype=np.float32),
        "b3": np.asarray(inputs["b3"], dtype=np.float32).reshape(1, NCLS),
    }
    in_maps = []
    for i in range(NCORES):
        sl = slice(i * BPC, (i + 1) * BPC)
        m = dict(shared)
        m["tokens"] = np.ascontiguousarray(tokens[sl])
        m["hidden"] = np.ascontiguousarray(hidden[sl])
        in_maps.append(m)

    res = run_bass_kernel_spmd(
        nc, in_maps, core_ids=list(range(NCORES)), trace=bool(_CACHED.get("trace")),
        tmpdir=_CACHED.get("tmpdir"),
    )
    _CACHED["last_res"] = res
    out = np.concatenate([res.results[i]["out"] for i in range(NCORES)], axis=0)
    return out.astype(np.float32)


if __name__ == "__main__":
    # quick self-test against a numpy reference
    rng = np.random.default_rng(0)
    ins = {
        "tokens": rng.integers(0, 100, (B, S)).astype(np.int64),
        "hidden_states": rng.standard_normal((B, S, H)).astype(np.float32),
        "proj_w": (rng.standard_normal((H, CIT)) / np.sqrt(H)).astype(np.float32),
        "proj_b": (rng.standard_normal(CIT) * 0.02).astype(np.float32),
        "enc_w": (rng.standard_normal((CIT, CIT)) / np.sqrt(CIT)).astype(np.float32),
        "enc_b": (rng.standard_normal(CIT) * 0.02).astype(np.float32),
        "w1": (rng.standard_normal((D1, D2)) / np.sqrt(D1)).astype(np.float32),
        "b1": (rng.standard_normal(D2) * 0.02).astype(np.float32),
        "w2": (rng.standard_normal((D2, D2)) / np.sqrt(D2)).astype(np.float32),
        "b2": (rng.standard_normal(D2) * 0.02).astype(np.float32),
        "w3": (rng.standard_normal((D2, NCLS)) / np.sqrt(D2)).astype(np.float32),
        "b3": (rng.standard_normal(NCLS) * 0.02).astype(np.float32),
    }
    got = kernel(**ins)
    print("kernel out", got.shape, got.dtype, got[:2])



# revision 6
# speedup vs baseline: 1.7342x; 1.7342x over previous
"""Trainium2 Bass kernel for nn_CitationClassifier (pooling/ridge).

Strategy: pure data parallel over the batch dim (256 = 8 cores x 32),
bf16 end-to-end (tolerance is 2e-2; bf16 keeps us ~10x under it):
  - hidden_states cast to bf16 on host -> halves the dominant DMA stream
  - token scan (find '@' span, CITSEG pos) on DVE in f32 math
  - span-masked max-pool via fused scalar_tensor_tensor chain
    (acc = (h_chunk + mask_c) max acc), PE-transpose + free-dim max
    reduce -> pooled^T [768, 32] bf16
  - CITSEG row gather via indirect DMA, projection + encoder GEMMs
    (feature-major, weights stationary), all bf16 with f32 PSUM accum
  - 3-layer MLP batch-major (x stationary, weights moving) in bf16;
    w1 resident in SBUF, w2 streamed with deep prefetch
Output [32, 6] f32 per core, concatenated on host to [256, 6].
"""

import sys

for _p in ("/opt/trn_rl_repo", "/root/.axon_site/_ro/trn_rl_repo"):
    if _p not in sys.path:
        sys.path.insert(0, _p)

import numpy as np

# --- problem dims (hardcoded per harness contract) ---
B, S, H = 256, 512, 768
CIT, D1, D2, NCLS = 750, 1518, 3036, 6
NCORES = 8
BPC = B // NCORES  # 32 samples per core
P = 128
AT_ID, CITSEG_ID = 5, 7
NEG = -3.0e38  # large negative fill, exactly representable in bf16

_CACHED = {}


def _build_bass():
    from concourse import bacc, bass, mybir
    import concourse.tile as tile
    from concourse.masks import make_identity

    dt = mybir.dt
    op = mybir.AluOpType
    act = mybir.ActivationFunctionType
    ax = mybir.AxisListType

    f32, i32, bf = dt.float32, dt.int32, dt.bfloat16

    nc = bacc.Bacc("TRN2", target_bir_lowering=False, debug=False)

    # ---- DRAM parameters (per-core shard shapes) ----
    tokens_d = nc.declare_dram_parameter("tokens", [BPC, S], i32, isOutput=False)
    hidden_d = nc.declare_dram_parameter("hidden", [BPC, S, H], bf, isOutput=False)
    projw_d = nc.declare_dram_parameter("proj_w", [H, CIT], bf, isOutput=False)
    projb_d = nc.declare_dram_parameter("proj_b", [1, CIT], bf, isOutput=False)
    encw_d = nc.declare_dram_parameter("enc_w", [CIT, CIT], bf, isOutput=False)
    encb_d = nc.declare_dram_parameter("enc_b", [1, CIT], bf, isOutput=False)
    w1_d = nc.declare_dram_parameter("w1", [D1, D2], bf, isOutput=False)
    b1_d = nc.declare_dram_parameter("b1", [1, D2], bf, isOutput=False)
    w2_d = nc.declare_dram_parameter("w2", [D2, D2], bf, isOutput=False)
    b2_d = nc.declare_dram_parameter("b2", [1, D2], bf, isOutput=False)
    w3_d = nc.declare_dram_parameter("w3", [D2, NCLS], bf, isOutput=False)
    b3_d = nc.declare_dram_parameter("b3", [1, NCLS], bf, isOutput=False)
    out_d = nc.declare_dram_parameter("out", [BPC, NCLS], f32, isOutput=True)

    def ktiles(total):
        ts, off = [], 0
        while off < total:
            sz = min(P, total - off)
            ts.append((off, sz))
            off += sz
        return ts

    KT_X = ktiles(D1)    # 12: 11x128 + 110
    KT_H = ktiles(D2)    # 24: 23x128 + 92
    KT_HID = ktiles(H)   # 6x128
    KT_CIT = ktiles(CIT)  # 5x128 + 110
    # psum column chunks, each within one 512-f32 bank
    CH512 = [(c, min(512, D2 - c)) for c in range(0, D2, 512)]

    with tile.TileContext(nc) as tc:
        with (
            tc.tile_pool(name="consts", bufs=1) as cpool,
            tc.tile_pool(name="hb", bufs=3) as hbp,
            tc.tile_pool(name="mx", bufs=2) as mxp,
            tc.tile_pool(name="wmov", bufs=8) as wp,
            tc.tile_pool(name="psptr", bufs=2, space="PSUM") as psp,
            tc.tile_pool(name="psbig", bufs=1, space="PSUM") as psb,
            tc.tile_pool(name="pssmall", bufs=2, space="PSUM") as pss,
            tc.tile_pool(name="psbf", bufs=1, space="PSUM") as pbf,
        ):
            # ---------- constants ----------
            identf = cpool.tile([P, P], f32, tag="identf")
            make_identity(nc, identf[:])
            identb = cpool.tile([P, P], bf, tag="identb")
            make_identity(nc, identb[:])
            ones_col = cpool.tile([1, BPC], bf, tag="ones_col")
            nc.vector.memset(ones_col[:], 1.0)
            ones_row = cpool.tile([1, P], bf, tag="ones_row")
            nc.vector.memset(ones_row[:], 1.0)

            # ---------- phase 0: token scan (f32 math, small) ----------
            tok_i = cpool.tile([BPC, S], i32, tag="tok_i")
            nc.sync.dma_start(out=tok_i[:], in_=tokens_d[:])
            tok = cpool.tile([BPC, S], f32, tag="tok")
            nc.vector.tensor_copy(out=tok[:], in_=tok_i[:])

            iota_i = cpool.tile([BPC, S], i32, tag="iota_i")
            nc.gpsimd.iota(iota_i[:], pattern=[[1, S]], base=0, channel_multiplier=0)
            iot = cpool.tile([BPC, S], f32, tag="iot")
            nc.vector.tensor_copy(out=iot[:], in_=iota_i[:])

            biota_i = cpool.tile([BPC, 1], i32, tag="biota_i")
            nc.gpsimd.iota(biota_i[:], pattern=[[0, 1]], base=0, channel_multiplier=1)
            biota = cpool.tile([BPC, 1], f32, tag="biota")
            nc.vector.tensor_copy(out=biota[:], in_=biota_i[:])

            def ts_(out_ap, in_ap, s1, o1, s2=None, o2=op.bypass):
                nc.vector.tensor_scalar(
                    out=out_ap, in0=in_ap, scalar1=s1, scalar2=s2, op0=o1, op1=o2
                )

            def tt_(out_ap, a, b_, o):
                nc.vector.tensor_tensor(out=out_ap, in0=a, in1=b_, op=o)

            def bc(ap_, shape):
                return ap_.to_broadcast(shape)

            t_a = cpool.tile([BPC, S], f32, tag="t_a")
            t_b = cpool.tile([BPC, S], f32, tag="t_b")
            t_c = cpool.tile([BPC, S], f32, tag="t_c")
            s_1 = cpool.tile([BPC, 1], f32, tag="s_1")
            first = cpool.tile([BPC, 1], f32, tag="first")
            second = cpool.tile([BPC, 1], f32, tag="second")
            ge2 = cpool.tile([BPC, 1], f32, tag="ge2")
            start = cpool.tile([BPC, 1], f32, tag="start")
            end = cpool.tile([BPC, 1], f32, tag="end")
            keep = cpool.tile([BPC, S], f32, tag="keep")
            keepany = cpool.tile([BPC, 1], f32, tag="keepany")
            maskneg = cpool.tile([BPC, S], f32, tag="maskneg")
            hasc = cpool.tile([BPC, 1], f32, tag="hasc")
            spos = cpool.tile([BPC, 1], f32, tag="spos")
            gidx_f = cpool.tile([BPC, 1], f32, tag="gidx_f")
            gidx_i = cpool.tile([BPC, 1], i32, tag="gidx_i")

            # at = (tok == 5); first/second occurrence, span, keep mask
            ts_(t_c[:], tok[:], float(AT_ID), op.is_equal)
            ts_(t_a[:], t_c[:], -1000.0, op.mult, 1000.0, op.add)
            tt_(t_a[:], t_a[:], iot[:], op.add)
            nc.vector.tensor_reduce(out=first[:], in_=t_a[:], axis=ax.X, op=op.min)
            tt_(t_b[:], iot[:], bc(first[:], [BPC, S]), op.is_gt)
            tt_(t_b[:], t_b[:], t_c[:], op.mult)
            ts_(t_b[:], t_b[:], -1000.0, op.mult, 1000.0, op.add)
            tt_(t_b[:], t_b[:], iot[:], op.add)
            nc.vector.tensor_reduce(out=second[:], in_=t_b[:], axis=ax.X, op=op.min)
            nc.vector.tensor_reduce(out=s_1[:], in_=t_c[:], axis=ax.X, op=op.add)
            ts_(ge2[:], s_1[:], 2.0, op.is_ge)
            tt_(start[:], first[:], ge2[:], op.mult)
            ts_(end[:], second[:], -float(S), op.add)
            tt_(end[:], end[:], ge2[:], op.mult)
            ts_(end[:], end[:], float(S), op.add)
            tt_(t_a[:], iot[:], bc(start[:], [BPC, S]), op.is_lt)
            tt_(t_b[:], iot[:], bc(end[:], [BPC, S]), op.is_gt)
            tt_(keep[:], t_a[:], t_b[:], op.max)
            nc.vector.tensor_reduce(out=keepany[:], in_=keep[:], axis=ax.X, op=op.max)
            # maskneg = (keep-1)*NEG -> 0 where keep, NEG where masked
            ts_(maskneg[:], keep[:], -1.0, op.add, -NEG, op.mult)
            # CITSEG first occurrence
            ts_(t_c[:], tok[:], float(CITSEG_ID), op.is_equal)
            ts_(t_a[:], t_c[:], -1000.0, op.mult, 1000.0, op.add)
            tt_(t_a[:], t_a[:], iot[:], op.add)
            nc.vector.tensor_reduce(out=s_1[:], in_=t_a[:], axis=ax.X, op=op.min)
            ts_(hasc[:], s_1[:], float(S - 1), op.is_le)
            ts_(spos[:], s_1[:], float(S - 1), op.min)
            ts_(gidx_f[:], biota[:], float(S), op.mult)
            tt_(gidx_f[:], gidx_f[:], spos[:], op.add)
            nc.vector.tensor_copy(out=gidx_i[:], in_=gidx_f[:])

            # ---------- weight DMAs (issued early, overlap everything) ----------
            w1sb = cpool.tile([P, len(KT_X), D2], bf, tag="w1sb")
            nf1 = D1 // P  # 11
            nc.sync.dma_start(
                out=w1sb[:, 0:nf1, :],
                in_=w1_d[0:nf1 * P, :].rearrange("(t p) m -> p t m", p=P),
            )
            nc.sync.dma_start(out=w1sb[0:D1 - nf1 * P, nf1, :], in_=w1_d[nf1 * P:D1, :])
            projw_sb = cpool.tile([P, len(KT_HID), CIT], bf, tag="projw_sb")
            nc.sync.dma_start(
                out=projw_sb[:], in_=projw_d[:].rearrange("(t p) m -> p t m", p=P)
            )
            encw_sb = cpool.tile([P, len(KT_CIT), CIT], bf, tag="encw_sb")
            nc.vector.memset(encw_sb[:], 0.0)
            nfc = CIT // P  # 5
            nc.sync.dma_start(
                out=encw_sb[:, 0:nfc, :],
                in_=encw_d[0:nfc * P, :].rearrange("(t p) m -> p t m", p=P),
            )
            nc.sync.dma_start(out=encw_sb[0:CIT - nfc * P, nfc, :], in_=encw_d[nfc * P:CIT, :])
            w3sb = cpool.tile([P, len(KT_H), NCLS], bf, tag="w3sb")
            nf3 = D2 // P  # 23
            nc.sync.dma_start(
                out=w3sb[:, 0:nf3, :],
                in_=w3_d[0:nf3 * P, :].rearrange("(t p) m -> p t m", p=P),
            )
            nc.sync.dma_start(out=w3sb[0:D2 - nf3 * P, nf3, :], in_=w3_d[nf3 * P:D2, :])
            projb_sb = cpool.tile([1, CIT], bf, tag="projb_sb")
            nc.sync.dma_start(out=projb_sb[:], in_=projb_d[:])
            encb_sb = cpool.tile([1, CIT], bf, tag="encb_sb")
            nc.sync.dma_start(out=encb_sb[:], in_=encb_d[:])
            b1_sb = cpool.tile([1, D2], bf, tag="b1_sb")
            nc.sync.dma_start(out=b1_sb[:], in_=b1_d[:])
            b2_sb = cpool.tile([1, D2], bf, tag="b2_sb")
            nc.sync.dma_start(out=b2_sb[:], in_=b2_d[:])
            b3_sb = cpool.tile([1, NCLS], bf, tag="b3_sb")
            nc.sync.dma_start(out=b3_sb[:], in_=b3_d[:])

            # ---------- masks transposed to [128 s, chunk, 32 b] (bf16) ----------
            maskcols = cpool.tile([P, 4, BPC], f32, tag="maskcols")
            for c in range(4):
                pt = pss.tile([P, BPC], f32, tag="pt_small")
                nc.tensor.transpose(
                    out=pt[:], in_=maskneg[:, c * P:(c + 1) * P],
                    identity=identf[:BPC, :BPC],
                )
                nc.vector.tensor_copy(out=maskcols[:, c, :], in_=pt[:])

            hasc_row = cpool.tile([1, BPC], bf, tag="hasc_row")
            pt = pss.tile([P, BPC], f32, tag="pt_small")
            nc.tensor.transpose(out=pt[:1, :], in_=hasc[:], identity=identf[:BPC, :BPC])
            nc.vector.tensor_copy(out=hasc_row[:], in_=pt[:1, :])

            ka_row = cpool.tile([1, BPC], bf, tag="ka_row")
            pt = pss.tile([P, BPC], f32, tag="pt_small")
            nc.tensor.transpose(out=pt[:1, :], in_=keepany[:], identity=identf[:BPC, :BPC])
            nc.vector.tensor_copy(out=ka_row[:], in_=pt[:1, :])
            kab = cpool.tile([P, BPC], bf, tag="kab")
            pt = pss.tile([P, BPC], f32, tag="pt_small")
            nc.tensor.matmul(out=pt[:], lhsT=ones_row[:], rhs=ka_row[:], start=True, stop=True)
            nc.vector.tensor_copy(out=kab[:], in_=pt[:])

            # ---------- CITSEG gather + cit_h^T + proj + enc ----------
            cith = cpool.tile([BPC, H], bf, tag="cith")
            hid_flat = hidden_d[:].rearrange("b s h -> (b s) h")
            nc.gpsimd.indirect_dma_start(
                out=cith[:],
                out_offset=None,
                in_=hid_flat,
                in_offset=bass.IndirectOffsetOnAxis(ap=gidx_i[:, :1], axis=0),
            )
            hasc_b = cpool.tile([BPC, 1], bf, tag="hasc_b")
            nc.vector.tensor_copy(out=hasc_b[:], in_=hasc[:])
            tt_(cith[:], cith[:], bc(hasc_b[:], [BPC, H]), op.mult)
            cithT = cpool.tile([P, len(KT_HID), BPC], bf, tag="cithT")
            for t in range(len(KT_HID)):
                pt = pbf.tile([P, BPC], bf, tag="pt_bf")
                nc.tensor.transpose(
                    out=pt[:], in_=cith[:, t * P:(t + 1) * P], identity=identb[:BPC, :BPC]
                )
                nc.vector.tensor_copy(out=cithT[:, t, :], in_=pt[:])

            # xT holds x^T = [pooled; cit_enc]^T as 12 k-tiles of [128, 32]
            xT = cpool.tile([P, len(KT_X), BPC], bf, tag="xT")

            # proj (feature-major, weights stationary)
            cpT = cpool.tile([P, len(KT_CIT), BPC], bf, tag="cpT")
            for mt, (moff, msz) in enumerate(KT_CIT):
                ps = pss.tile([P, BPC], f32, tag="pt_small")
                for kt in range(len(KT_HID)):
                    nc.tensor.matmul(
                        out=ps[:msz, :],
                        lhsT=projw_sb[:, kt, moff:moff + msz],
                        rhs=cithT[:, kt, :],
                        start=(kt == 0), stop=False,
                    )
                nc.tensor.matmul(
                    out=ps[:msz, :], lhsT=projb_sb[:, moff:moff + msz],
                    rhs=hasc_row[:], start=False, stop=True,
                )
                nc.vector.tensor_copy(out=cpT[:msz, mt, :], in_=ps[:msz, :])

            # encoder (feature-major)
            for mt, (moff, msz) in enumerate(KT_CIT):
                ps = pss.tile([P, BPC], f32, tag="pt_small")
                for kt, (koff, ksz) in enumerate(KT_CIT):
                    nc.tensor.matmul(
                        out=ps[:msz, :],
                        lhsT=encw_sb[:ksz, kt, moff:moff + msz],
                        rhs=cpT[:ksz, kt, :],
                        start=(kt == 0), stop=False,
                    )
                nc.tensor.matmul(
                    out=ps[:msz, :], lhsT=encb_sb[:, moff:moff + msz],
                    rhs=ones_col[:], start=False, stop=True,
                )
                nc.vector.tensor_copy(out=xT[:msz, 6 + mt, :], in_=ps[:msz, :])

            # ---------- pooling over S (per sample, fused mask+max chain) ----------
            for b in range(BPC):
                hb = hbp.tile([P, 4, H], bf, tag="hb")
                nc.sync.dma_start(
                    out=hb[:], in_=hidden_d[b].rearrange("(c p) h -> p c h", p=P)
                )
                acc = mxp.tile([P, H], bf, tag="acc")
                # acc = hb[:,0,:] + m0 ; then acc = (hb[:,c,:] + mc) max acc
                nc.vector.tensor_scalar(
                    out=acc[:], in0=hb[:, 0, :], scalar1=maskcols[:, 0, b:b + 1],
                    scalar2=None, op0=op.add,
                )
                for c in range(1, 4):
                    nc.vector.scalar_tensor_tensor(
                        out=acc[:], in0=hb[:, c, :], scalar=maskcols[:, c, b:b + 1],
                        in1=acc[:], op0=op.add, op1=op.max,
                    )
                # transpose h-chunks to psum; reduce over s-lanes
                ptr = psp.tile([P, 6, P], bf, tag="ptr")
                for t in range(6):
                    nc.tensor.transpose(
                        out=ptr[:, t, :], in_=acc[:, t * P:(t + 1) * P],
                        identity=identb[:],
                    )
                nc.vector.tensor_reduce(
                    out=xT[:, 0:6, b], in_=ptr[:], axis=ax.X, op=op.max,
                )
            # zero pooled where no kept position
            for t in range(6):
                tt_(xT[:, t, :], xT[:, t, :], kab[:], op.mult)

            # ---------- MLP (batch-major: x stationary, weights moving) ----------
            HW2 = D2 // 2  # 1518: two column passes, psum [32, 1518] = 3 banks
            CH = [(0, 512), (512, 512), (1024, HW2 - 1024)]
            h1 = cpool.tile([BPC, D2], bf, tag="h1")
            for poff in (0, HW2):
                psg = psb.tile([BPC, HW2], f32, tag="psg")
                for kt, (koff, ksz) in enumerate(KT_X):
                    for (c0, cw) in CH:
                        nc.tensor.matmul(
                            out=psg[:, c0:c0 + cw],
                            lhsT=xT[:ksz, kt, :],
                            rhs=w1sb[:ksz, kt, poff + c0:poff + c0 + cw],
                            start=(kt == 0), stop=False,
                        )
                for (c0, cw) in CH:
                    nc.tensor.matmul(
                        out=psg[:, c0:c0 + cw], lhsT=ones_col[:],
                        rhs=b1_sb[:, poff + c0:poff + c0 + cw], start=False, stop=True,
                    )
                nc.scalar.activation(out=h1[:, poff:poff + HW2], in_=psg[:], func=act.Relu)

            h1T = cpool.tile([P, len(KT_H), BPC], bf, tag="h1T")
            for t, (toff, tsz) in enumerate(KT_H):
                pt = pbf.tile([P, BPC], bf, tag="pt_bf")
                nc.tensor.transpose(
                    out=pt[:tsz, :], in_=h1[:, toff:toff + tsz], identity=identb[:BPC, :BPC]
                )
                nc.vector.tensor_copy(out=h1T[:tsz, t, :], in_=pt[:tsz, :])

            h2 = cpool.tile([BPC, D2], bf, tag="h2")
            for poff in (0, HW2):
                psg2 = psb.tile([BPC, HW2], f32, tag="psg")
                for kt, (koff, ksz) in enumerate(KT_H):
                    wt = wp.tile([P, HW2], bf, tag="wt")
                    nc.sync.dma_start(
                        out=wt[:ksz, :], in_=w2_d[koff:koff + ksz, poff:poff + HW2]
                    )
                    for (c0, cw) in CH:
                        nc.tensor.matmul(
                            out=psg2[:, c0:c0 + cw],
                            lhsT=h1T[:ksz, kt, :],
                            rhs=wt[:ksz, c0:c0 + cw],
                            start=(kt == 0), stop=False,
                        )
                for (c0, cw) in CH:
                    nc.tensor.matmul(
                        out=psg2[:, c0:c0 + cw], lhsT=ones_col[:],
                        rhs=b2_sb[:, poff + c0:poff + c0 + cw], start=False, stop=True,
                    )
                nc.scalar.activation(out=h2[:, poff:poff + HW2], in_=psg2[:], func=act.Relu)

            h2T = cpool.tile([P, len(KT_H), BPC], bf, tag="h2T")
            for t, (toff, tsz) in enumerate(KT_H):
                pt = pbf.tile([P, BPC], bf, tag="pt_bf")
                nc.tensor.transpose(
                    out=pt[:tsz, :], in_=h2[:, toff:toff + tsz], identity=identb[:BPC, :BPC]
                )
                nc.vector.tensor_copy(out=h2T[:tsz, t, :], in_=pt[:tsz, :])

            pso = pss.tile([P, BPC], f32, tag="pt_small")
            for kt, (koff, ksz) in enumerate(KT_H):
                nc.tensor.matmul(
                    out=pso[:BPC, :NCLS],
                    lhsT=h2T[:ksz, kt, :],
                    rhs=w3sb[:ksz, kt, :],
                    start=(kt == 0), stop=False,
                )
            nc.tensor.matmul(
                out=pso[:BPC, :NCLS], lhsT=ones_col[:],
                rhs=b3_sb[:], start=False, stop=True,
            )
            out_sb = cpool.tile([BPC, NCLS], f32, tag="out_sb")
            nc.vector.tensor_copy(out=out_sb[:], in_=pso[:BPC, :NCLS])
            nc.sync.dma_start(out=out_d[:], in_=out_sb[:])

    nc.compile()
    return nc


def _get_nc():
    if "nc" not in _CACHED:
        _CACHED["nc"] = _build_bass()
    return _CACHED["nc"]


def kernel(**inputs) -> np.ndarray:
    from concourse.bass_utils import run_bass_kernel_spmd
    import ml_dtypes

    bfl = ml_dtypes.bfloat16
    nc = _get_nc()

    tokens = np.asarray(inputs["tokens"]).astype(np.int32)
    hidden = np.ascontiguousarray(
        np.asarray(inputs["hidden_states"], dtype=np.float32).astype(bfl)
    )

    def wcast(name, shape=None):
        a = np.asarray(inputs[name], dtype=np.float32).astype(bfl)
        if shape is not None:
            a = a.reshape(shape)
        return np.ascontiguousarray(a)

    shared = {
        "proj_w": wcast("proj_w"),
        "proj_b": wcast("proj_b", (1, CIT)),
        "enc_w": wcast("enc_w"),
        "enc_b": wcast("enc_b", (1, CIT)),
        "w1": wcast("w1"),
        "b1": wcast("b1", (1, D2)),
        "w2": wcast("w2"),
        "b2": wcast("b2", (1, D2)),
        "w3": wcast("w3"),
        "b3": wcast("b3", (1, NCLS)),
    }
    in_maps = []
    for i in range(NCORES):
        sl = slice(i * BPC, (i + 1) * BPC)
        m = dict(shared)
        m["tokens"] = np.ascontiguousarray(tokens[sl])
        m["hidden"] = np.ascontiguousarray(hidden[sl])
        in_maps.append(m)

    res = run_bass_kernel_spmd(
        nc, in_maps, core_ids=list(range(NCORES)), trace=bool(_CACHED.get("trace")),
        tmpdir=_CACHED.get("tmpdir"),
    )
    _CACHED["last_res"] = res
    out = np.concatenate([res.results[i]["out"] for i in range(NCORES)], axis=0)
    return out.astype(np.float32)


if __name__ == "__main__":
    # quick self-test against a numpy reference
    rng = np.random.default_rng(0)
    ins = {
        "tokens": rng.integers(0, 100, (B, S)).astype(np.int64),
        "hidden_states": rng.standard_normal((B, S, H)).astype(np.float32),
        "proj_w": (rng.standard_normal((H, CIT)) / np.sqrt(H)).astype(np.float32),
        "proj_b": (rng.standard_normal(CIT) * 0.02).astype(np.float32),
        "enc_w": (rng.standard_normal((CIT, CIT)) / np.sqrt(CIT)).astype(np.float32),
        "enc_b": (rng.standard_normal(CIT) * 0.02).astype(np.float32),
        "w1": (rng.standard_normal((D1, D2)) / np.sqrt(D1)).astype(np.float32),
        "b1": (rng.standard_normal(D2) * 0.02).astype(np.float32),
        "w2": (rng.standard_normal((D2, D2)) / np.sqrt(D2)).astype(np.float32),
        "b2": (rng.standard_normal(D2) * 0.02).astype(np.float32),
        "w3": (rng.standard_normal((D2, NCLS)) / np.sqrt(D2)).astype(np.float32),
        "b3": (rng.standard_normal(NCLS) * 0.02).astype(np.float32),
    }
    got = kernel(**ins)
    print("kernel out", got.shape, got.dtype, got[:2])


# revision 10
# speedup vs baseline: 2.1499x; 1.2397x over previous
"""Trainium2 Bass kernel for nn_CitationClassifier (pooling/ridge).

Data parallel over batch (256 = 8 cores x 32), bf16 end-to-end.
v2: pooling split across ACT/GPSIMD/DVE, host pre-tiled weights
(contiguous per-partition DMA lines), p-major hidden layout,
MLP with PE column-tiling (4 batch-groups packed into [128,768] PSUM),
single-pass w2 streaming in 2-ktile chunks.
"""

import sys

for _p in ("/opt/trn_rl_repo", "/root/.axon_site/_ro/trn_rl_repo"):
    if _p not in sys.path:
        sys.path.insert(0, _p)

import numpy as np

B, S, H = 256, 512, 768
CIT, D1, D2, NCLS = 750, 1518, 3036, 6
NCORES = 8
BPC = B // NCORES  # 32
P = 128
AT_ID, CITSEG_ID = 5, 7
NEG = -3.0e38

NKT_X = 12   # ceil(1518/128)
NKT_H = 24   # ceil(3036/128)
NKT_HID = 6
NKT_CIT = 6
NW2 = 12     # w2 streamed in 12 chunks of 2 k-tiles

_CACHED = {}


def _build_bass():
    from concourse import bacc, bass, mybir
    import concourse.tile as tile
    from concourse.masks import make_identity

    dt = mybir.dt
    op = mybir.AluOpType
    act = mybir.ActivationFunctionType
    ax = mybir.AxisListType

    f32, i32, bf = dt.float32, dt.int32, dt.bfloat16

    nc = bacc.Bacc("TRN2", target_bir_lowering=False, debug=False)

    tokens_d = nc.declare_dram_parameter("tokens", [BPC, S], i32, isOutput=False)
    hidden_d = nc.declare_dram_parameter("hidden", [BPC, S, H], bf, isOutput=False)
    projw_d = nc.declare_dram_parameter("proj_w", [P, NKT_HID, CIT], bf, isOutput=False)
    projb_d = nc.declare_dram_parameter("proj_b", [1, CIT], bf, isOutput=False)
    encw_d = nc.declare_dram_parameter("enc_w", [P, NKT_CIT, CIT], bf, isOutput=False)
    encb_d = nc.declare_dram_parameter("enc_b", [1, CIT], bf, isOutput=False)
    w1_d = nc.declare_dram_parameter("w1", [P, NKT_X, D2], bf, isOutput=False)
    b1_d = nc.declare_dram_parameter("b1", [1, D2], bf, isOutput=False)
    w2_d = nc.declare_dram_parameter("w2", [NW2, P, 2, D2], bf, isOutput=False)
    b2_d = nc.declare_dram_parameter("b2", [1, D2], bf, isOutput=False)
    w3_d = nc.declare_dram_parameter("w3", [P, NKT_H, NCLS], bf, isOutput=False)
    b3_d = nc.declare_dram_parameter("b3", [1, NCLS], bf, isOutput=False)
    out_d = nc.declare_dram_parameter("out", [BPC, NCLS], f32, isOutput=True)

    def ktiles(total, n):
        return [(i * P, min(P, total - i * P)) for i in range(n)]

    KT_X = ktiles(D1, NKT_X)
    KT_H = ktiles(D2, NKT_H)
    KT_CIT = ktiles(CIT, NKT_CIT)
    # 4 batch-groups packed into psum partitions via PE column tiling
    GRP = [(g, 768 * g, min(768, D2 - 768 * g)) for g in range(4)]  # widths 768,768,768,732

    with tile.TileContext(nc) as tc:
        with (
            tc.tile_pool(name="consts", bufs=1) as cpool,
            tc.tile_pool(name="hb", bufs=3) as hbp,
            tc.tile_pool(name="mx", bufs=2) as mxp,
            tc.tile_pool(name="wmov", bufs=3) as wp,
            tc.tile_pool(name="psptr", bufs=2, space="PSUM") as psp,
            tc.tile_pool(name="psbig", bufs=2, space="PSUM") as psb,
            tc.tile_pool(name="pssmall", bufs=2, space="PSUM") as pss,
        ):
            # ---------- constants ----------
            identf = cpool.tile([P, P], f32, tag="identf")
            make_identity(nc, identf[:])
            identb = cpool.tile([P, P], bf, tag="identb")
            make_identity(nc, identb[:])
            ones_col = cpool.tile([1, BPC], bf, tag="ones_col")
            nc.vector.memset(ones_col[:], 1.0)
            ones_row = cpool.tile([1, P], bf, tag="ones_row")
            nc.vector.memset(ones_row[:], 1.0)

            # ---------- token scan (f32, small) ----------
            tok_i = cpool.tile([BPC, S], i32, tag="tok_i")
            nc.sync.dma_start(out=tok_i[:], in_=tokens_d[:])
            tok = cpool.tile([BPC, S], f32, tag="tok")
            nc.vector.tensor_copy(out=tok[:], in_=tok_i[:])

            iota_i = cpool.tile([BPC, S], i32, tag="iota_i")
            nc.gpsimd.iota(iota_i[:], pattern=[[1, S]], base=0, channel_multiplier=0)
            iot = cpool.tile([BPC, S], f32, tag="iot")
            nc.vector.tensor_copy(out=iot[:], in_=iota_i[:])

            biota_i = cpool.tile([BPC, 1], i32, tag="biota_i")
            nc.gpsimd.iota(biota_i[:], pattern=[[0, 1]], base=0, channel_multiplier=1)
            biota = cpool.tile([BPC, 1], f32, tag="biota")
            nc.vector.tensor_copy(out=biota[:], in_=biota_i[:])

            def ts_(out_ap, in_ap, s1, o1, s2=None, o2=op.bypass):
                nc.vector.tensor_scalar(
                    out=out_ap, in0=in_ap, scalar1=s1, scalar2=s2, op0=o1, op1=o2
                )

            def tt_(out_ap, a, b_, o):
                nc.vector.tensor_tensor(out=out_ap, in0=a, in1=b_, op=o)

            def bc(ap_, shape):
                return ap_.to_broadcast(shape)

            t_a = cpool.tile([BPC, S], f32, tag="t_a")
            t_b = cpool.tile([BPC, S], f32, tag="t_b")
            t_c = cpool.tile([BPC, S], f32, tag="t_c")
            s_1 = cpool.tile([BPC, 1], f32, tag="s_1")
            first = cpool.tile([BPC, 1], f32, tag="first")
            second = cpool.tile([BPC, 1], f32, tag="second")
            ge2 = cpool.tile([BPC, 1], f32, tag="ge2")
            start = cpool.tile([BPC, 1], f32, tag="start")
            end = cpool.tile([BPC, 1], f32, tag="end")
            keep = cpool.tile([BPC, S], f32, tag="keep")
            keepany = cpool.tile([BPC, 1], f32, tag="keepany")
            maskneg = cpool.tile([BPC, S], f32, tag="maskneg")
            hasc = cpool.tile([BPC, 1], f32, tag="hasc")
            spos = cpool.tile([BPC, 1], f32, tag="spos")
            gidx_f = cpool.tile([BPC, 1], f32, tag="gidx_f")
            gidx_i = cpool.tile([BPC, 1], i32, tag="gidx_i")

            ts_(t_c[:], tok[:], float(AT_ID), op.is_equal)
            ts_(t_a[:], t_c[:], -1000.0, op.mult, 1000.0, op.add)
            tt_(t_a[:], t_a[:], iot[:], op.add)
            nc.vector.tensor_reduce(out=first[:], in_=t_a[:], axis=ax.X, op=op.min)
            tt_(t_b[:], iot[:], bc(first[:], [BPC, S]), op.is_gt)
            tt_(t_b[:], t_b[:], t_c[:], op.mult)
            ts_(t_b[:], t_b[:], -1000.0, op.mult, 1000.0, op.add)
            tt_(t_b[:], t_b[:], iot[:], op.add)
            nc.vector.tensor_reduce(out=second[:], in_=t_b[:], axis=ax.X, op=op.min)
            nc.vector.tensor_reduce(out=s_1[:], in_=t_c[:], axis=ax.X, op=op.add)
            ts_(ge2[:], s_1[:], 2.0, op.is_ge)
            tt_(start[:], first[:], ge2[:], op.mult)
            ts_(end[:], second[:], -float(S), op.add)
            tt_(end[:], end[:], ge2[:], op.mult)
            ts_(end[:], end[:], float(S), op.add)
            tt_(t_a[:], iot[:], bc(start[:], [BPC, S]), op.is_lt)
            tt_(t_b[:], iot[:], bc(end[:], [BPC, S]), op.is_gt)
            tt_(keep[:], t_a[:], t_b[:], op.max)
            nc.vector.tensor_reduce(out=keepany[:], in_=keep[:], axis=ax.X, op=op.max)
            ts_(maskneg[:], keep[:], -1.0, op.add, -NEG, op.mult)
            ts_(t_c[:], tok[:], float(CITSEG_ID), op.is_equal)
            ts_(t_a[:], t_c[:], -1000.0, op.mult, 1000.0, op.add)
            tt_(t_a[:], t_a[:], iot[:], op.add)
            nc.vector.tensor_reduce(out=s_1[:], in_=t_a[:], axis=ax.X, op=op.min)
            ts_(hasc[:], s_1[:], float(S - 1), op.is_le)
            ts_(spos[:], s_1[:], float(S - 1), op.min)
            ts_(gidx_f[:], biota[:], float(S), op.mult)
            tt_(gidx_f[:], gidx_f[:], spos[:], op.add)
            nc.vector.tensor_copy(out=gidx_i[:], in_=gidx_f[:])

            # ---------- weight DMAs (host pre-tiled, contiguous lines) ----------
            w1sb = cpool.tile([P, NKT_X, D2], bf, tag="w1sb")
            nc.sync.dma_start(out=w1sb[:], in_=w1_d[:])
            projw_sb = cpool.tile([P, NKT_HID, CIT], bf, tag="projw_sb")
            nc.sync.dma_start(out=projw_sb[:], in_=projw_d[:])
            encw_sb = cpool.tile([P, NKT_CIT, CIT], bf, tag="encw_sb")
            nc.sync.dma_start(out=encw_sb[:], in_=encw_d[:])
            w3sb = cpool.tile([P, NKT_H, NCLS], bf, tag="w3sb")
            nc.sync.dma_start(out=w3sb[:], in_=w3_d[:])
            projb_sb = cpool.tile([1, CIT], bf, tag="projb_sb")
            nc.sync.dma_start(out=projb_sb[:], in_=projb_d[:])
            encb_sb = cpool.tile([1, CIT], bf, tag="encb_sb")
            nc.sync.dma_start(out=encb_sb[:], in_=encb_d[:])
            b1_sb = cpool.tile([1, D2], bf, tag="b1_sb")
            nc.sync.dma_start(out=b1_sb[:], in_=b1_d[:])
            b2_sb = cpool.tile([1, D2], bf, tag="b2_sb")
            nc.sync.dma_start(out=b2_sb[:], in_=b2_d[:])
            b3_sb = cpool.tile([1, NCLS], bf, tag="b3_sb")
            nc.sync.dma_start(out=b3_sb[:], in_=b3_d[:])

            # ---------- masks -> [128 p, 4 c, 32 b] f32 (s = 4p + c) ----------
            mrearr = maskneg[:].rearrange("b (p c) -> b p c", c=4)
            mscr = cpool.tile([BPC, 4, P], f32, tag="mscr")
            for c in range(4):
                nc.vector.tensor_copy(out=mscr[:, c, :], in_=mrearr[:, :, c])
            maskcols = cpool.tile([P, 4, BPC], f32, tag="maskcols")
            for c in range(4):
                pt = pss.tile([P, BPC], f32, tag="pt_small")
                nc.tensor.transpose(
                    out=pt[:], in_=mscr[:, c, :], identity=identf[:BPC, :BPC]
                )
                nc.vector.tensor_copy(out=maskcols[:, c, :], in_=pt[:])

            hasc_row = cpool.tile([1, BPC], bf, tag="hasc_row")
            pt = pss.tile([P, BPC], f32, tag="pt_small")
            nc.tensor.transpose(out=pt[:1, :], in_=hasc[:], identity=identf[:BPC, :BPC])
            nc.vector.tensor_copy(out=hasc_row[:], in_=pt[:1, :])

            ka_row = cpool.tile([1, BPC], bf, tag="ka_row")
            pt = pss.tile([P, BPC], f32, tag="pt_small")
            nc.tensor.transpose(out=pt[:1, :], in_=keepany[:], identity=identf[:BPC, :BPC])
            nc.vector.tensor_copy(out=ka_row[:], in_=pt[:1, :])
            kab = cpool.tile([P, BPC], bf, tag="kab")
            pt = pss.tile([P, BPC], f32, tag="pt_small")
            nc.tensor.matmul(out=pt[:], lhsT=ones_row[:], rhs=ka_row[:], start=True, stop=True)
            nc.vector.tensor_copy(out=kab[:], in_=pt[:])

            # ---------- CITSEG gather + proj + enc (feature-major) ----------
            cith = cpool.tile([BPC, H], bf, tag="cith")
            hid_flat = hidden_d[:].rearrange("b s h -> (b s) h")
            nc.gpsimd.indirect_dma_start(
                out=cith[:],
                out_offset=None,
                in_=hid_flat,
                in_offset=bass.IndirectOffsetOnAxis(ap=gidx_i[:, :1], axis=0),
            )
            hasc_b = cpool.tile([BPC, 1], bf, tag="hasc_b")
            nc.vector.tensor_copy(out=hasc_b[:], in_=hasc[:])
            tt_(cith[:], cith[:], bc(hasc_b[:], [BPC, H]), op.mult)
            cithT = cpool.tile([P, NKT_HID, BPC], bf, tag="cithT")
            for t in range(NKT_HID):
                pt = psp.tile([P, 6, P], bf, tag="ptr")
                nc.tensor.transpose(
                    out=pt[:, 0, :BPC], in_=cith[:, t * P:(t + 1) * P],
                    identity=identb[:BPC, :BPC],
                )
                nc.vector.tensor_copy(out=cithT[:, t, :], in_=pt[:, 0, :BPC])

            xT = cpool.tile([P, NKT_X, BPC], bf, tag="xT")

            cpT = cpool.tile([P, NKT_CIT, BPC], bf, tag="cpT")
            for mt, (moff, msz) in enumerate(KT_CIT):
                ps = pss.tile([P, BPC], f32, tag="pt_small")
                for kt in range(NKT_HID):
                    nc.tensor.matmul(
                        out=ps[:msz, :],
                        lhsT=projw_sb[:, kt, moff:moff + msz],
                        rhs=cithT[:, kt, :],
                        start=(kt == 0), stop=False,
                    )
                nc.tensor.matmul(
                    out=ps[:msz, :], lhsT=projb_sb[:, moff:moff + msz],
                    rhs=hasc_row[:], start=False, stop=True,
                )
                nc.vector.tensor_copy(out=cpT[:msz, mt, :], in_=ps[:msz, :])

            for mt, (moff, msz) in enumerate(KT_CIT):
                ps = pss.tile([P, BPC], f32, tag="pt_small")
                for kt, (koff, ksz) in enumerate(KT_CIT):
                    nc.tensor.matmul(
                        out=ps[:msz, :],
                        lhsT=encw_sb[:ksz, kt, moff:moff + msz],
                        rhs=cpT[:ksz, kt, :],
                        start=(kt == 0), stop=False,
                    )
                nc.tensor.matmul(
                    out=ps[:msz, :], lhsT=encb_sb[:, moff:moff + msz],
                    rhs=ones_col[:], start=False, stop=True,
                )
                nc.vector.tensor_copy(out=xT[:msz, 6 + mt, :], in_=ps[:msz, :])

            # ---------- pooling: ACT 3 masked adds, GPS 1 max, DVE rest ----------
            for b in range(BPC):
                hb = hbp.tile([P, 4, H], bf, tag="hb")
                nc.sync.dma_start(
                    out=hb[:], in_=hidden_d[b].rearrange("(p c) h -> p c h", p=P)
                )
                m1 = mxp.tile([P, H], bf, tag="m1")
                m2 = mxp.tile([P, H], bf, tag="m2")
                m3 = mxp.tile([P, H], bf, tag="m3")
                acc = mxp.tile([P, H], bf, tag="acc")
                nc.scalar.activation(
                    out=m1[:], in_=hb[:, 1, :], func=act.Identity,
                    bias=maskcols[:, 1, b:b + 1], scale=1.0,
                )
                nc.scalar.activation(
                    out=m2[:], in_=hb[:, 2, :], func=act.Identity,
                    bias=maskcols[:, 2, b:b + 1], scale=1.0,
                )
                nc.scalar.activation(
                    out=m3[:], in_=hb[:, 3, :], func=act.Identity,
                    bias=maskcols[:, 3, b:b + 1], scale=1.0,
                )
                nc.vector.tensor_scalar(
                    out=acc[:], in0=hb[:, 0, :], scalar1=maskcols[:, 0, b:b + 1],
                    scalar2=None, op0=op.add,
                )
                tt_(acc[:], acc[:], m1[:], op.max)
                tt_(acc[:], acc[:], m2[:], op.max)
                tt_(acc[:], acc[:], m3[:], op.max)
                ptr = psp.tile([P, 6, P], bf, tag="ptr")
                for t in range(6):
                    nc.tensor.transpose(
                        out=ptr[:, t, :], in_=acc[:, t * P:(t + 1) * P],
                        identity=identb[:],
                    )
                nc.vector.tensor_reduce(
                    out=xT[:, 0:6, b], in_=ptr[:], axis=ax.X, op=op.max,
                )
            for t in range(6):
                tt_(xT[:, t, :], xT[:, t, :], kab[:], op.mult)

            # ---------- MLP: batch-major, 4 groups col-tiled into [128,768] ----------
            def mlp_layer(kt_list, lhsT_fn, rhs_fn, bias_sb, h_out):
                psg = psb.tile([P, 768], f32, tag="psg")
                for kt, (koff, ksz) in enumerate(kt_list):
                    for (g, goff, gw) in GRP:
                        for (c0, cw) in ((0, 512), (512, gw - 512)):
                            nc.tensor.matmul(
                                out=psg[32 * g:32 * g + 32, c0:c0 + cw],
                                lhsT=lhsT_fn(kt, ksz),
                                rhs=rhs_fn(kt, ksz, goff + c0, cw),
                                start=(kt == 0), stop=False,
                                tile_position=(0, 32 * g),
                            )
                for (g, goff, gw) in GRP:
                    for (c0, cw) in ((0, 512), (512, gw - 512)):
                        nc.tensor.matmul(
                            out=psg[32 * g:32 * g + 32, c0:c0 + cw],
                            lhsT=ones_col[:],
                            rhs=bias_sb[:, goff + c0:goff + c0 + cw],
                            start=False, stop=True,
                            tile_position=(0, 32 * g),
                        )
                nc.scalar.activation(out=h_out[:], in_=psg[:], func=act.Relu)

            def transpose_groups(h_in, hT_out):
                # h_in [128, 768]: partition b+32g holds features 768g + j
                for kt, (koff, ksz) in enumerate(KT_H):
                    g, t = kt // 6, kt % 6
                    pt = psp.tile([P, 6, P], bf, tag="ptr")
                    nc.tensor.transpose(
                        out=pt[:ksz, 0, :BPC],
                        in_=h_in[32 * g:32 * g + 32, t * P:t * P + ksz],
                        identity=identb[32 * g:32 * g + 32, 32 * g:32 * g + 32],
                        tile_position=(32 * g, 0),
                    )
                    nc.vector.tensor_copy(out=hT_out[:ksz, kt, :], in_=pt[:ksz, 0, :BPC])

            h1 = cpool.tile([P, 768], bf, tag="h1")
            mlp_layer(
                KT_X,
                lambda kt, ksz: xT[:ksz, kt, :],
                lambda kt, ksz, c0, cw: w1sb[:ksz, kt, c0:c0 + cw],
                b1_sb, h1,
            )
            h1T = cpool.tile([P, NKT_H, BPC], bf, tag="h1T")
            transpose_groups(h1, h1T)

            h2 = cpool.tile([P, 768], bf, tag="h2")
            psg = psb.tile([P, 768], f32, tag="psg")
            wt = None
            for kt, (koff, ksz) in enumerate(KT_H):
                if kt % 2 == 0:
                    wt = wp.tile([P, 2, D2], bf, tag="wt")
                    nc.sync.dma_start(out=wt[:], in_=w2_d[kt // 2])
                for (g, goff, gw) in GRP:
                    for (c0, cw) in ((0, 512), (512, gw - 512)):
                        nc.tensor.matmul(
                            out=psg[32 * g:32 * g + 32, c0:c0 + cw],
                            lhsT=h1T[:ksz, kt, :],
                            rhs=wt[:ksz, kt % 2, goff + c0:goff + c0 + cw],
                            start=(kt == 0), stop=False,
                            tile_position=(0, 32 * g),
                        )
            for (g, goff, gw) in GRP:
                for (c0, cw) in ((0, 512), (512, gw - 512)):
                    nc.tensor.matmul(
                        out=psg[32 * g:32 * g + 32, c0:c0 + cw],
                        lhsT=ones_col[:],
                        rhs=b2_sb[:, goff + c0:goff + c0 + cw],
                        start=False, stop=True,
                        tile_position=(0, 32 * g),
                    )
            nc.scalar.activation(out=h2[:], in_=psg[:], func=act.Relu)

            h2T = cpool.tile([P, NKT_H, BPC], bf, tag="h2T")
            transpose_groups(h2, h2T)

            pso = pss.tile([P, BPC], f32, tag="pt_small")
            for kt, (koff, ksz) in enumerate(KT_H):
                nc.tensor.matmul(
                    out=pso[:BPC, :NCLS],
                    lhsT=h2T[:ksz, kt, :],
                    rhs=w3sb[:ksz, kt, :],
                    start=(kt == 0), stop=False,
                )
            nc.tensor.matmul(
                out=pso[:BPC, :NCLS], lhsT=ones_col[:],
                rhs=b3_sb[:], start=False, stop=True,
            )
            out_sb = cpool.tile([BPC, NCLS], f32, tag="out_sb")
            nc.vector.tensor_copy(out=out_sb[:], in_=pso[:BPC, :NCLS])
            nc.sync.dma_start(out=out_d[:], in_=out_sb[:])

    nc.compile()
    return nc


def _get_nc():
    if "nc" not in _CACHED:
        _CACHED["nc"] = _build_bass()
    return _CACHED["nc"]


def _ktile_pack(w, n_kt, bfl):
    """[K, M] f32 -> [128, n_kt, M] bf16 with zero row padding."""
    K, M = w.shape
    wp_ = np.zeros((n_kt * 128, M), dtype=np.float32)
    wp_[:K] = w
    return np.ascontiguousarray(
        wp_.reshape(n_kt, 128, M).transpose(1, 0, 2).astype(bfl)
    )


def kernel(**inputs) -> np.ndarray:
    from concourse.bass_utils import run_bass_kernel_spmd
    import ml_dtypes

    bfl = ml_dtypes.bfloat16
    nc = _get_nc()

    tokens = np.asarray(inputs["tokens"]).astype(np.int32)
    hidden = np.ascontiguousarray(
        np.asarray(inputs["hidden_states"], dtype=np.float32).astype(bfl)
    )
    w2 = np.asarray(inputs["w2"], dtype=np.float32)
    w2p = np.zeros((NW2 * 256, D2), dtype=np.float32)
    w2p[:D2] = w2
    w2h = np.ascontiguousarray(
        w2p.reshape(NW2, 2, 128, D2).transpose(0, 2, 1, 3).astype(bfl)
    )

    def vcast(name, shape):
        return np.ascontiguousarray(
            np.asarray(inputs[name], dtype=np.float32).astype(bfl).reshape(shape)
        )

    shared = {
        "proj_w": _ktile_pack(np.asarray(inputs["proj_w"], np.float32), NKT_HID, bfl),
        "proj_b": vcast("proj_b", (1, CIT)),
        "enc_w": _ktile_pack(np.asarray(inputs["enc_w"], np.float32), NKT_CIT, bfl),
        "enc_b": vcast("enc_b", (1, CIT)),
        "w1": _ktile_pack(np.asarray(inputs["w1"], np.float32), NKT_X, bfl),
        "b1": vcast("b1", (1, D2)),
        "w2": w2h,
        "b2": vcast("b2", (1, D2)),
        "w3": _ktile_pack(np.asarray(inputs["w3"], np.float32), NKT_H, bfl),
        "b3": vcast("b3", (1, NCLS)),
    }
    in_maps = []
    for i in range(NCORES):
        sl = slice(i * BPC, (i + 1) * BPC)
        m = dict(shared)
        m["tokens"] = np.ascontiguousarray(tokens[sl])
        m["hidden"] = np.ascontiguousarray(hidden[sl])
        in_maps.append(m)

    res = run_bass_kernel_spmd(
        nc, in_maps, core_ids=list(range(NCORES)), trace=bool(_CACHED.get("trace")),
        tmpdir=_CACHED.get("tmpdir"),
    )
    _CACHED["last_res"] = res
    out = np.concatenate([res.results[i]["out"] for i in range(NCORES)], axis=0)
    return out.astype(np.float32)


if __name__ == "__main__":
    rng = np.random.default_rng(0)
    ins = {
        "tokens": rng.integers(0, 100, (B, S)).astype(np.int64),
        "hidden_states": rng.standard_normal((B, S, H)).astype(np.float32),
        "proj_w": (rng.standard_normal((H, CIT)) / np.sqrt(H)).astype(np.float32),
        "proj_b": (rng.standard_normal(CIT) * 0.02).astype(np.float32),
        "enc_w": (rng.standard_normal((CIT, CIT)) / np.sqrt(CIT)).astype(np.float32),
        "enc_b": (rng.standard_normal(CIT) * 0.02).astype(np.float32),
        "w1": (rng.standard_normal((D1, D2)) / np.sqrt(D1)).astype(np.float32),
        "b1": (rng.standard_normal(D2) * 0.02).astype(np.float32),
        "w2": (rng.standard_normal((D2, D2)) / np.sqrt(D2)).astype(np.float32),
        "b2": (rng.standard_normal(D2) * 0.02).astype(np.float32),
        "w3": (rng.standard_normal((D2, NCLS)) / np.sqrt(D2)).astype(np.float32),
        "b3": (rng.standard_normal(NCLS) * 0.02).astype(np.float32),
    }
    got = kernel(**ins)
    print("kernel out", got.shape, got.dtype, got[:2])


# revision 11
# speedup vs baseline: 2.3445x; 1.0905x over previous
"""Trainium2 Bass kernel for nn_CitationClassifier (pooling/ridge).

Data parallel over batch (256 = 8 cores x 32), bf16 end-to-end.
v2: pooling split across ACT/GPSIMD/DVE, host pre-tiled weights
(contiguous per-partition DMA lines), p-major hidden layout,
MLP with PE column-tiling (4 batch-groups packed into [128,768] PSUM),
single-pass w2 streaming in 2-ktile chunks.
"""

import sys

for _p in ("/opt/trn_rl_repo", "/root/.axon_site/_ro/trn_rl_repo"):
    if _p not in sys.path:
        sys.path.insert(0, _p)

import numpy as np

B, S, H = 256, 512, 768
CIT, D1, D2, NCLS = 750, 1518, 3036, 6
NCORES = 8
BPC = B // NCORES  # 32
P = 128
AT_ID, CITSEG_ID = 5, 7
NEG = -3.0e38

NKT_X = 12   # ceil(1518/128)
NKT_H = 24   # ceil(3036/128)
NKT_HID = 6
NKT_CIT = 6
NW2 = 12     # w2 streamed in 12 chunks of 2 k-tiles

_CACHED = {}


def _build_bass():
    from concourse import bacc, bass, mybir
    import concourse.tile as tile
    from concourse.masks import make_identity

    dt = mybir.dt
    op = mybir.AluOpType
    act = mybir.ActivationFunctionType
    ax = mybir.AxisListType

    f32, i32, bf = dt.float32, dt.int32, dt.bfloat16

    nc = bacc.Bacc("TRN2", target_bir_lowering=False, debug=False)

    tokens_d = nc.declare_dram_parameter("tokens", [BPC, S], i32, isOutput=False)
    hidden_d = nc.declare_dram_parameter("hidden", [BPC, S, H], bf, isOutput=False)
    projw_d = nc.declare_dram_parameter("proj_w", [P, NKT_HID, CIT], bf, isOutput=False)
    projb_d = nc.declare_dram_parameter("proj_b", [1, CIT], bf, isOutput=False)
    encw_d = nc.declare_dram_parameter("enc_w", [P, NKT_CIT, CIT], bf, isOutput=False)
    encb_d = nc.declare_dram_parameter("enc_b", [1, CIT], bf, isOutput=False)
    w1_d = nc.declare_dram_parameter("w1", [6, P, 2, D2], bf, isOutput=False)
    b1_d = nc.declare_dram_parameter("b1", [1, D2], bf, isOutput=False)
    w2_d = nc.declare_dram_parameter("w2", [NW2, P, 2, D2], bf, isOutput=False)
    b2_d = nc.declare_dram_parameter("b2", [1, D2], bf, isOutput=False)
    w3_d = nc.declare_dram_parameter("w3", [P, NKT_H, NCLS], bf, isOutput=False)
    b3_d = nc.declare_dram_parameter("b3", [1, NCLS], bf, isOutput=False)
    out_d = nc.declare_dram_parameter("out", [BPC, NCLS], f32, isOutput=True)

    def ktiles(total, n):
        return [(i * P, min(P, total - i * P)) for i in range(n)]

    KT_X = ktiles(D1, NKT_X)
    KT_H = ktiles(D2, NKT_H)
    KT_CIT = ktiles(CIT, NKT_CIT)
    # 4 batch-groups packed into psum partitions via PE column tiling
    GRP = [(g, 768 * g, min(768, D2 - 768 * g)) for g in range(4)]  # widths 768,768,768,732

    with tile.TileContext(nc) as tc:
        with (
            tc.tile_pool(name="consts", bufs=1) as cpool,
            tc.tile_pool(name="hb", bufs=3) as hbp,
            tc.tile_pool(name="mx", bufs=2) as mxp,
            tc.tile_pool(name="wmov", bufs=8) as wp,
            tc.tile_pool(name="psptr", bufs=2, space="PSUM") as psp,
            tc.tile_pool(name="psbig", bufs=2, space="PSUM") as psb,
            tc.tile_pool(name="pssmall", bufs=2, space="PSUM") as pss,
        ):
            # ---------- constants ----------
            identf = cpool.tile([P, P], f32, tag="identf")
            make_identity(nc, identf[:])
            identb = cpool.tile([P, P], bf, tag="identb")
            make_identity(nc, identb[:])
            ones_col = cpool.tile([1, BPC], bf, tag="ones_col")
            nc.vector.memset(ones_col[:], 1.0)
            ones_row = cpool.tile([1, P], bf, tag="ones_row")
            nc.vector.memset(ones_row[:], 1.0)

            # ---------- token scan (f32, small) ----------
            tok_i = cpool.tile([BPC, S], i32, tag="tok_i")
            nc.sync.dma_start(out=tok_i[:], in_=tokens_d[:])
            tok = cpool.tile([BPC, S], f32, tag="tok")
            nc.vector.tensor_copy(out=tok[:], in_=tok_i[:])

            iota_i = cpool.tile([BPC, S], i32, tag="iota_i")
            nc.gpsimd.iota(iota_i[:], pattern=[[1, S]], base=0, channel_multiplier=0)
            iot = cpool.tile([BPC, S], f32, tag="iot")
            nc.vector.tensor_copy(out=iot[:], in_=iota_i[:])

            biota_i = cpool.tile([BPC, 1], i32, tag="biota_i")
            nc.gpsimd.iota(biota_i[:], pattern=[[0, 1]], base=0, channel_multiplier=1)
            biota = cpool.tile([BPC, 1], f32, tag="biota")
            nc.vector.tensor_copy(out=biota[:], in_=biota_i[:])

            def ts_(out_ap, in_ap, s1, o1, s2=None, o2=op.bypass):
                nc.vector.tensor_scalar(
                    out=out_ap, in0=in_ap, scalar1=s1, scalar2=s2, op0=o1, op1=o2
                )

            def tt_(out_ap, a, b_, o):
                nc.vector.tensor_tensor(out=out_ap, in0=a, in1=b_, op=o)

            def bc(ap_, shape):
                return ap_.to_broadcast(shape)

            t_a = cpool.tile([BPC, S], f32, tag="t_a")
            t_b = cpool.tile([BPC, S], f32, tag="t_b")
            t_c = cpool.tile([BPC, S], f32, tag="t_c")
            s_1 = cpool.tile([BPC, 1], f32, tag="s_1")
            first = cpool.tile([BPC, 1], f32, tag="first")
            second = cpool.tile([BPC, 1], f32, tag="second")
            ge2 = cpool.tile([BPC, 1], f32, tag="ge2")
            start = cpool.tile([BPC, 1], f32, tag="start")
            end = cpool.tile([BPC, 1], f32, tag="end")
            keep = cpool.tile([BPC, S], f32, tag="keep")
            keepany = cpool.tile([BPC, 1], f32, tag="keepany")
            maskneg = cpool.tile([BPC, S], f32, tag="maskneg")
            hasc = cpool.tile([BPC, 1], f32, tag="hasc")
            spos = cpool.tile([BPC, 1], f32, tag="spos")
            gidx_f = cpool.tile([BPC, 1], f32, tag="gidx_f")
            gidx_i = cpool.tile([BPC, 1], i32, tag="gidx_i")

            ts_(t_c[:], tok[:], float(AT_ID), op.is_equal)
            ts_(t_a[:], t_c[:], -1000.0, op.mult, 1000.0, op.add)
            tt_(t_a[:], t_a[:], iot[:], op.add)
            nc.vector.tensor_reduce(out=first[:], in_=t_a[:], axis=ax.X, op=op.min)
            tt_(t_b[:], iot[:], bc(first[:], [BPC, S]), op.is_gt)
            tt_(t_b[:], t_b[:], t_c[:], op.mult)
            ts_(t_b[:], t_b[:], -1000.0, op.mult, 1000.0, op.add)
            tt_(t_b[:], t_b[:], iot[:], op.add)
            nc.vector.tensor_reduce(out=second[:], in_=t_b[:], axis=ax.X, op=op.min)
            nc.vector.tensor_reduce(out=s_1[:], in_=t_c[:], axis=ax.X, op=op.add)
            ts_(ge2[:], s_1[:], 2.0, op.is_ge)
            tt_(start[:], first[:], ge2[:], op.mult)
            ts_(end[:], second[:], -float(S), op.add)
            tt_(end[:], end[:], ge2[:], op.mult)
            ts_(end[:], end[:], float(S), op.add)
            tt_(t_a[:], iot[:], bc(start[:], [BPC, S]), op.is_lt)
            tt_(t_b[:], iot[:], bc(end[:], [BPC, S]), op.is_gt)
            tt_(keep[:], t_a[:], t_b[:], op.max)
            nc.vector.tensor_reduce(out=keepany[:], in_=keep[:], axis=ax.X, op=op.max)
            ts_(maskneg[:], keep[:], -1.0, op.add, -NEG, op.mult)
            ts_(t_c[:], tok[:], float(CITSEG_ID), op.is_equal)
            ts_(t_a[:], t_c[:], -1000.0, op.mult, 1000.0, op.add)
            tt_(t_a[:], t_a[:], iot[:], op.add)
            nc.vector.tensor_reduce(out=s_1[:], in_=t_a[:], axis=ax.X, op=op.min)
            ts_(hasc[:], s_1[:], float(S - 1), op.is_le)
            ts_(spos[:], s_1[:], float(S - 1), op.min)
            ts_(gidx_f[:], biota[:], float(S), op.mult)
            tt_(gidx_f[:], gidx_f[:], spos[:], op.add)
            nc.vector.tensor_copy(out=gidx_i[:], in_=gidx_f[:])

            # ---------- weight DMAs (host pre-tiled, contiguous lines) ----------
            projw_sb = cpool.tile([P, NKT_HID, CIT], bf, tag="projw_sb")
            nc.sync.dma_start(out=projw_sb[:], in_=projw_d[:])
            encw_sb = cpool.tile([P, NKT_CIT, CIT], bf, tag="encw_sb")
            nc.sync.dma_start(out=encw_sb[:], in_=encw_d[:])
            w3sb = cpool.tile([P, NKT_H, NCLS], bf, tag="w3sb")
            nc.sync.dma_start(out=w3sb[:], in_=w3_d[:])
            projb_sb = cpool.tile([1, CIT], bf, tag="projb_sb")
            nc.sync.dma_start(out=projb_sb[:], in_=projb_d[:])
            encb_sb = cpool.tile([1, CIT], bf, tag="encb_sb")
            nc.sync.dma_start(out=encb_sb[:], in_=encb_d[:])
            b1_sb = cpool.tile([1, D2], bf, tag="b1_sb")
            nc.sync.dma_start(out=b1_sb[:], in_=b1_d[:])
            b2_sb = cpool.tile([1, D2], bf, tag="b2_sb")
            nc.sync.dma_start(out=b2_sb[:], in_=b2_d[:])
            b3_sb = cpool.tile([1, NCLS], bf, tag="b3_sb")
            nc.sync.dma_start(out=b3_sb[:], in_=b3_d[:])

            # ---------- masks -> [128 p, 4 c, 32 b] f32 (s = 4p + c) ----------
            mrearr = maskneg[:].rearrange("b (p c) -> b p c", c=4)
            mscr = cpool.tile([BPC, 4, P], f32, tag="mscr")
            for c in range(4):
                nc.vector.tensor_copy(out=mscr[:, c, :], in_=mrearr[:, :, c])
            maskcols = cpool.tile([P, 4, BPC], f32, tag="maskcols")
            for c in range(4):
                pt = pss.tile([P, BPC], f32, tag="pt_small")
                nc.tensor.transpose(
                    out=pt[:], in_=mscr[:, c, :], identity=identf[:BPC, :BPC]
                )
                nc.vector.tensor_copy(out=maskcols[:, c, :], in_=pt[:])

            hasc_row = cpool.tile([1, BPC], bf, tag="hasc_row")
            pt = pss.tile([P, BPC], f32, tag="pt_small")
            nc.tensor.transpose(out=pt[:1, :], in_=hasc[:], identity=identf[:BPC, :BPC])
            nc.vector.tensor_copy(out=hasc_row[:], in_=pt[:1, :])

            ka_row = cpool.tile([1, BPC], bf, tag="ka_row")
            pt = pss.tile([P, BPC], f32, tag="pt_small")
            nc.tensor.transpose(out=pt[:1, :], in_=keepany[:], identity=identf[:BPC, :BPC])
            nc.vector.tensor_copy(out=ka_row[:], in_=pt[:1, :])
            kab = cpool.tile([P, BPC], bf, tag="kab")
            pt = pss.tile([P, BPC], f32, tag="pt_small")
            nc.tensor.matmul(out=pt[:], lhsT=ones_row[:], rhs=ka_row[:], start=True, stop=True)
            nc.vector.tensor_copy(out=kab[:], in_=pt[:])

            # ---------- CITSEG gather + proj + enc (feature-major) ----------
            cith = cpool.tile([BPC, H], bf, tag="cith")
            hid_flat = hidden_d[:].rearrange("b s h -> (b s) h")
            nc.gpsimd.indirect_dma_start(
                out=cith[:],
                out_offset=None,
                in_=hid_flat,
                in_offset=bass.IndirectOffsetOnAxis(ap=gidx_i[:, :1], axis=0),
            )
            hasc_b = cpool.tile([BPC, 1], bf, tag="hasc_b")
            nc.vector.tensor_copy(out=hasc_b[:], in_=hasc[:])
            tt_(cith[:], cith[:], bc(hasc_b[:], [BPC, H]), op.mult)
            cithT = cpool.tile([P, NKT_HID, BPC], bf, tag="cithT")
            for t in range(NKT_HID):
                pt = psp.tile([P, 6, P], bf, tag="ptr")
                nc.tensor.transpose(
                    out=pt[:, 0, :BPC], in_=cith[:, t * P:(t + 1) * P],
                    identity=identb[:BPC, :BPC],
                )
                nc.vector.tensor_copy(out=cithT[:, t, :], in_=pt[:, 0, :BPC])

            xT = cpool.tile([P, NKT_X, BPC], bf, tag="xT")

            cpT = cpool.tile([P, NKT_CIT, BPC], bf, tag="cpT")
            for mt, (moff, msz) in enumerate(KT_CIT):
                ps = pss.tile([P, BPC], f32, tag="pt_small")
                for kt in range(NKT_HID):
                    nc.tensor.matmul(
                        out=ps[:msz, :],
                        lhsT=projw_sb[:, kt, moff:moff + msz],
                        rhs=cithT[:, kt, :],
                        start=(kt == 0), stop=False,
                    )
                nc.tensor.matmul(
                    out=ps[:msz, :], lhsT=projb_sb[:, moff:moff + msz],
                    rhs=hasc_row[:], start=False, stop=True,
                )
                nc.vector.tensor_copy(out=cpT[:msz, mt, :], in_=ps[:msz, :])

            for mt, (moff, msz) in enumerate(KT_CIT):
                ps = pss.tile([P, BPC], f32, tag="pt_small")
                for kt, (koff, ksz) in enumerate(KT_CIT):
                    nc.tensor.matmul(
                        out=ps[:msz, :],
                        lhsT=encw_sb[:ksz, kt, moff:moff + msz],
                        rhs=cpT[:ksz, kt, :],
                        start=(kt == 0), stop=False,
                    )
                nc.tensor.matmul(
                    out=ps[:msz, :], lhsT=encb_sb[:, moff:moff + msz],
                    rhs=ones_col[:], start=False, stop=True,
                )
                nc.vector.tensor_copy(out=xT[:msz, 6 + mt, :], in_=ps[:msz, :])

            # ---------- pooling: ACT 3 masked adds, GPS 1 max, DVE rest ----------
            for b in range(BPC):
                hb = hbp.tile([P, 4, H], bf, tag="hb")
                nc.sync.dma_start(
                    out=hb[:], in_=hidden_d[b].rearrange("(p c) h -> p c h", p=P)
                )
                m1 = mxp.tile([P, H], bf, tag="m1")
                m2 = mxp.tile([P, H], bf, tag="m2")
                m3 = mxp.tile([P, H], bf, tag="m3")
                acc = mxp.tile([P, H], bf, tag="acc")
                nc.scalar.activation(
                    out=m1[:], in_=hb[:, 1, :], func=act.Identity,
                    bias=maskcols[:, 1, b:b + 1], scale=1.0,
                )
                nc.scalar.activation(
                    out=m2[:], in_=hb[:, 2, :], func=act.Identity,
                    bias=maskcols[:, 2, b:b + 1], scale=1.0,
                )
                nc.scalar.activation(
                    out=m3[:], in_=hb[:, 3, :], func=act.Identity,
                    bias=maskcols[:, 3, b:b + 1], scale=1.0,
                )
                nc.vector.tensor_scalar(
                    out=acc[:], in0=hb[:, 0, :], scalar1=maskcols[:, 0, b:b + 1],
                    scalar2=None, op0=op.add,
                )
                tt_(acc[:], acc[:], m1[:], op.max)
                tt_(acc[:], acc[:], m2[:], op.max)
                tt_(acc[:], acc[:], m3[:], op.max)
                ptr = psp.tile([P, 6, P], bf, tag="ptr")
                for t in range(6):
                    nc.tensor.transpose(
                        out=ptr[:, t, :], in_=acc[:, t * P:(t + 1) * P],
                        identity=identb[:],
                    )
                nc.vector.tensor_reduce(
                    out=xT[:, 0:6, b], in_=ptr[:], axis=ax.X, op=op.max,
                )
            for t in range(6):
                tt_(xT[:, t, :], xT[:, t, :], kab[:], op.mult)

            # ---------- MLP: batch-major, 4 groups col-tiled into [128,768] ----------
            def mlp_layer(kt_list, lhsT_fn, rhs_fn, bias_sb, h_out):
                psg = psb.tile([P, 768], f32, tag="psg")
                for kt, (koff, ksz) in enumerate(kt_list):
                    for (g, goff, gw) in GRP:
                        for (c0, cw) in ((0, 512), (512, gw - 512)):
                            nc.tensor.matmul(
                                out=psg[32 * g:32 * g + 32, c0:c0 + cw],
                                lhsT=lhsT_fn(kt, ksz),
                                rhs=rhs_fn(kt, ksz, goff + c0, cw),
                                start=(kt == 0), stop=False,
                                tile_position=(0, 32 * g),
                            )
                for (g, goff, gw) in GRP:
                    for (c0, cw) in ((0, 512), (512, gw - 512)):
                        nc.tensor.matmul(
                            out=psg[32 * g:32 * g + 32, c0:c0 + cw],
                            lhsT=ones_col[:],
                            rhs=bias_sb[:, goff + c0:goff + c0 + cw],
                            start=False, stop=True,
                            tile_position=(0, 32 * g),
                        )
                nc.scalar.activation(out=h_out[:], in_=psg[:], func=act.Relu)

            def transpose_groups(h_in, hT_out):
                # h_in [128, 768]: partition b+32g holds features 768g + j
                for kt, (koff, ksz) in enumerate(KT_H):
                    g, t = kt // 6, kt % 6
                    pt = psp.tile([P, 6, P], bf, tag="ptr")
                    nc.tensor.transpose(
                        out=pt[:ksz, 0, :BPC],
                        in_=h_in[32 * g:32 * g + 32, t * P:t * P + ksz],
                        identity=identb[32 * g:32 * g + 32, 32 * g:32 * g + 32],
                        tile_position=(32 * g, 0),
                    )
                    nc.vector.tensor_copy(out=hT_out[:ksz, kt, :], in_=pt[:ksz, 0, :BPC])

            h1 = cpool.tile([P, 768], bf, tag="h1")
            psg1 = psb.tile([P, 768], f32, tag="psg")
            wt1 = None
            for kt, (koff, ksz) in enumerate(KT_X):
                if kt % 2 == 0:
                    wt1 = wp.tile([P, 2, D2], bf, tag="wt")
                    nc.sync.dma_start(out=wt1[:], in_=w1_d[kt // 2])
                for (g, goff, gw) in GRP:
                    for (c0, cw) in ((0, 512), (512, gw - 512)):
                        nc.tensor.matmul(
                            out=psg1[32 * g:32 * g + 32, c0:c0 + cw],
                            lhsT=xT[:ksz, kt, :],
                            rhs=wt1[:ksz, kt % 2, goff + c0:goff + c0 + cw],
                            start=(kt == 0), stop=False,
                            tile_position=(0, 32 * g),
                        )
            for (g, goff, gw) in GRP:
                for (c0, cw) in ((0, 512), (512, gw - 512)):
                    nc.tensor.matmul(
                        out=psg1[32 * g:32 * g + 32, c0:c0 + cw],
                        lhsT=ones_col[:],
                        rhs=b1_sb[:, goff + c0:goff + c0 + cw],
                        start=False, stop=True,
                        tile_position=(0, 32 * g),
                    )
            nc.scalar.activation(out=h1[:], in_=psg1[:], func=act.Relu)
            h1T = cpool.tile([P, NKT_H, BPC], bf, tag="h1T")
            transpose_groups(h1, h1T)

            h2 = cpool.tile([P, 768], bf, tag="h2")
            psg = psb.tile([P, 768], f32, tag="psg")
            wt = None
            for kt, (koff, ksz) in enumerate(KT_H):
                if kt % 2 == 0:
                    wt = wp.tile([P, 2, D2], bf, tag="wt")
                    nc.sync.dma_start(out=wt[:], in_=w2_d[kt // 2])
                for (g, goff, gw) in GRP:
                    for (c0, cw) in ((0, 512), (512, gw - 512)):
                        nc.tensor.matmul(
                            out=psg[32 * g:32 * g + 32, c0:c0 + cw],
                            lhsT=h1T[:ksz, kt, :],
                            rhs=wt[:ksz, kt % 2, goff + c0:goff + c0 + cw],
                            start=(kt == 0), stop=False,
                            tile_position=(0, 32 * g),
                        )
            for (g, goff, gw) in GRP:
                for (c0, cw) in ((0, 512), (512, gw - 512)):
                    nc.tensor.matmul(
                        out=psg[32 * g:32 * g + 32, c0:c0 + cw],
                        lhsT=ones_col[:],
                        rhs=b2_sb[:, goff + c0:goff + c0 + cw],
                        start=False, stop=True,
                        tile_position=(0, 32 * g),
                    )
            nc.scalar.activation(out=h2[:], in_=psg[:], func=act.Relu)

            h2T = cpool.tile([P, NKT_H, BPC], bf, tag="h2T")
            transpose_groups(h2, h2T)

            pso = pss.tile([P, BPC], f32, tag="pt_small")
            for kt, (koff, ksz) in enumerate(KT_H):
                nc.tensor.matmul(
                    out=pso[:BPC, :NCLS],
                    lhsT=h2T[:ksz, kt, :],
                    rhs=w3sb[:ksz, kt, :],
                    start=(kt == 0), stop=False,
                )
            nc.tensor.matmul(
                out=pso[:BPC, :NCLS], lhsT=ones_col[:],
                rhs=b3_sb[:], start=False, stop=True,
            )
            out_sb = cpool.tile([BPC, NCLS], f32, tag="out_sb")
            nc.vector.tensor_copy(out=out_sb[:], in_=pso[:BPC, :NCLS])
            nc.sync.dma_start(out=out_d[:], in_=out_sb[:])

    nc.compile()
    return nc


def _get_nc():
    if "nc" not in _CACHED:
        _CACHED["nc"] = _build_bass()
    return _CACHED["nc"]


def _ktile_pack(w, n_kt, bfl):
    """[K, M] f32 -> [128, n_kt, M] bf16 with zero row padding."""
    K, M = w.shape
    wp_ = np.zeros((n_kt * 128, M), dtype=np.float32)
    wp_[:K] = w
    return np.ascontiguousarray(
        wp_.reshape(n_kt, 128, M).transpose(1, 0, 2).astype(bfl)
    )


def _chunk_pack(w, n_ch, bfl):
    """[K, M] f32 -> [n_ch, 128, 2, M] bf16, 2 k-tiles per chunk, zero pad."""
    K, M = w.shape
    wp_ = np.zeros((n_ch * 256, M), dtype=np.float32)
    wp_[:K] = w
    return np.ascontiguousarray(
        wp_.reshape(n_ch, 2, 128, M).transpose(0, 2, 1, 3).astype(bfl)
    )


def kernel(**inputs) -> np.ndarray:
    from concourse.bass_utils import run_bass_kernel_spmd
    import ml_dtypes

    bfl = ml_dtypes.bfloat16
    nc = _get_nc()

    tokens = np.asarray(inputs["tokens"]).astype(np.int32)
    hidden = np.ascontiguousarray(
        np.asarray(inputs["hidden_states"], dtype=np.float32).astype(bfl)
    )
    w2h = _chunk_pack(np.asarray(inputs["w2"], dtype=np.float32), NW2, bfl)

    def vcast(name, shape):
        return np.ascontiguousarray(
            np.asarray(inputs[name], dtype=np.float32).astype(bfl).reshape(shape)
        )

    shared = {
        "proj_w": _ktile_pack(np.asarray(inputs["proj_w"], np.float32), NKT_HID, bfl),
        "proj_b": vcast("proj_b", (1, CIT)),
        "enc_w": _ktile_pack(np.asarray(inputs["enc_w"], np.float32), NKT_CIT, bfl),
        "enc_b": vcast("enc_b", (1, CIT)),
        "w1": _chunk_pack(np.asarray(inputs["w1"], np.float32), 6, bfl),
        "b1": vcast("b1", (1, D2)),
        "w2": w2h,
        "b2": vcast("b2", (1, D2)),
        "w3": _ktile_pack(np.asarray(inputs["w3"], np.float32), NKT_H, bfl),
        "b3": vcast("b3", (1, NCLS)),
    }
    in_maps = []
    for i in range(NCORES):
        sl = slice(i * BPC, (i + 1) * BPC)
        m = dict(shared)
        m["tokens"] = np.ascontiguousarray(tokens[sl])
        m["hidden"] = np.ascontiguousarray(hidden[sl])
        in_maps.append(m)

    res = run_bass_kernel_spmd(
        nc, in_maps, core_ids=list(range(NCORES)), trace=bool(_CACHED.get("trace")),
        tmpdir=_CACHED.get("tmpdir"),
    )
    _CACHED["last_res"] = res
    out = np.concatenate([res.results[i]["out"] for i in range(NCORES)], axis=0)
    return out.astype(np.float32)


if __name__ == "__main__":
    rng = np.random.default_rng(0)
    ins = {
        "tokens": rng.integers(0, 100, (B, S)).astype(np.int64),
        "hidden_states": rng.standard_normal((B, S, H)).astype(np.float32),
        "proj_w": (rng.standard_normal((H, CIT)) / np.sqrt(H)).astype(np.float32),
        "proj_b": (rng.standard_normal(CIT) * 0.02).astype(np.float32),
        "enc_w": (rng.standard_normal((CIT, CIT)) / np.sqrt(CIT)).astype(np.float32),
        "enc_b": (rng.standard_normal(CIT) * 0.02).astype(np.float32),
        "w1": (rng.standard_normal((D1, D2)) / np.sqrt(D1)).astype(np.float32),
        "b1": (rng.standard_normal(D2) * 0.02).astype(np.float32),
        "w2": (rng.standard_normal((D2, D2)) / np.sqrt(D2)).astype(np.float32),
        "b2": (rng.standard_normal(D2) * 0.02).astype(np.float32),
        "w3": (rng.standard_normal((D2, NCLS)) / np.sqrt(D2)).astype(np.float32),
        "b3": (rng.standard_normal(NCLS) * 0.02).astype(np.float32),
    }
    got = kernel(**ins)
    print("kernel out", got.shape, got.dtype, got[:2])


# revision 12
# speedup vs baseline: 2.4718x; 1.0543x over previous
"""Trainium2 Bass kernel for nn_CitationClassifier (pooling/ridge).

Data parallel over batch (256 = 8 cores x 32), bf16 end-to-end.
v2: pooling split across ACT/GPSIMD/DVE, host pre-tiled weights
(contiguous per-partition DMA lines), p-major hidden layout,
MLP with PE column-tiling (4 batch-groups packed into [128,768] PSUM),
single-pass w2 streaming in 2-ktile chunks.
"""

import sys

for _p in ("/opt/trn_rl_repo", "/root/.axon_site/_ro/trn_rl_repo"):
    if _p not in sys.path:
        sys.path.insert(0, _p)

import numpy as np

B, S, H = 256, 512, 768
CIT, D1, D2, NCLS = 750, 1518, 3036, 6
NCORES = 8
BPC = B // NCORES  # 32
P = 128
AT_ID, CITSEG_ID = 5, 7
NEG = -3.0e38

NKT_X = 12   # ceil(1518/128)
NKT_H = 24   # ceil(3036/128)
NKT_HID = 6
NKT_CIT = 6
NW2 = 12     # w2 streamed in 12 chunks of 2 k-tiles

_CACHED = {}


def _build_bass():
    from concourse import bacc, bass, mybir
    import concourse.tile as tile
    from concourse.masks import make_identity

    dt = mybir.dt
    op = mybir.AluOpType
    act = mybir.ActivationFunctionType
    ax = mybir.AxisListType

    f32, i32, bf = dt.float32, dt.int32, dt.bfloat16

    nc = bacc.Bacc("TRN2", target_bir_lowering=False, debug=False)

    tokens_d = nc.declare_dram_parameter("tokens", [BPC, S], i32, isOutput=False)
    hidden_d = nc.declare_dram_parameter("hidden", [BPC, S, H], bf, isOutput=False)
    projw_d = nc.declare_dram_parameter("proj_w", [P, NKT_HID, CIT], bf, isOutput=False)
    projb_d = nc.declare_dram_parameter("proj_b", [1, CIT], bf, isOutput=False)
    encw_d = nc.declare_dram_parameter("enc_w", [P, NKT_CIT, CIT], bf, isOutput=False)
    encb_d = nc.declare_dram_parameter("enc_b", [1, CIT], bf, isOutput=False)
    w1_d = nc.declare_dram_parameter("w1", [6, P, 2, D2], bf, isOutput=False)
    b1_d = nc.declare_dram_parameter("b1", [1, D2], bf, isOutput=False)
    w2_d = nc.declare_dram_parameter("w2", [NW2, P, 2, D2], bf, isOutput=False)
    b2_d = nc.declare_dram_parameter("b2", [1, D2], bf, isOutput=False)
    w3_d = nc.declare_dram_parameter("w3", [P, NKT_H, NCLS], bf, isOutput=False)
    b3_d = nc.declare_dram_parameter("b3", [1, NCLS], bf, isOutput=False)
    out_d = nc.declare_dram_parameter("out", [BPC, NCLS], f32, isOutput=True)

    def ktiles(total, n):
        return [(i * P, min(P, total - i * P)) for i in range(n)]

    KT_X = ktiles(D1, NKT_X)
    KT_H = ktiles(D2, NKT_H)
    KT_CIT = ktiles(CIT, NKT_CIT)
    # 4 batch-groups packed into psum partitions via PE column tiling
    GRP = [(g, 768 * g, min(768, D2 - 768 * g)) for g in range(4)]  # widths 768,768,768,732

    with tile.TileContext(nc) as tc:
        with (
            tc.tile_pool(name="consts", bufs=1) as cpool,
            tc.tile_pool(name="hb", bufs=4) as hbp,
            tc.tile_pool(name="mx", bufs=3) as mxp,
            tc.tile_pool(name="wmov", bufs=8) as wp,
            tc.tile_pool(name="psptr", bufs=2, space="PSUM") as psp,
            tc.tile_pool(name="psbig", bufs=2, space="PSUM") as psb,
            tc.tile_pool(name="pssmall", bufs=2, space="PSUM") as pss,
        ):
            # ---------- constants ----------
            identf = cpool.tile([P, P], f32, tag="identf")
            make_identity(nc, identf[:])
            identb = cpool.tile([P, P], bf, tag="identb")
            make_identity(nc, identb[:])
            ones_col = cpool.tile([1, BPC], bf, tag="ones_col")
            nc.vector.memset(ones_col[:], 1.0)
            ones_row = cpool.tile([1, P], bf, tag="ones_row")
            nc.vector.memset(ones_row[:], 1.0)

            # ---------- token scan (f32, small) ----------
            tok_i = cpool.tile([BPC, S], i32, tag="tok_i")
            nc.sync.dma_start(out=tok_i[:], in_=tokens_d[:])
            tok = cpool.tile([BPC, S], f32, tag="tok")
            nc.vector.tensor_copy(out=tok[:], in_=tok_i[:])

            iota_i = cpool.tile([BPC, S], i32, tag="iota_i")
            nc.gpsimd.iota(iota_i[:], pattern=[[1, S]], base=0, channel_multiplier=0)
            iot = cpool.tile([BPC, S], f32, tag="iot")
            nc.vector.tensor_copy(out=iot[:], in_=iota_i[:])

            biota_i = cpool.tile([BPC, 1], i32, tag="biota_i")
            nc.gpsimd.iota(biota_i[:], pattern=[[0, 1]], base=0, channel_multiplier=1)
            biota = cpool.tile([BPC, 1], f32, tag="biota")
            nc.vector.tensor_copy(out=biota[:], in_=biota_i[:])

            def ts_(out_ap, in_ap, s1, o1, s2=None, o2=op.bypass):
                nc.vector.tensor_scalar(
                    out=out_ap, in0=in_ap, scalar1=s1, scalar2=s2, op0=o1, op1=o2
                )

            def tt_(out_ap, a, b_, o):
                nc.vector.tensor_tensor(out=out_ap, in0=a, in1=b_, op=o)

            def bc(ap_, shape):
                return ap_.to_broadcast(shape)

            t_a = cpool.tile([BPC, S], f32, tag="t_a")
            t_b = cpool.tile([BPC, S], f32, tag="t_b")
            t_c = cpool.tile([BPC, S], f32, tag="t_c")
            s_1 = cpool.tile([BPC, 1], f32, tag="s_1")
            first = cpool.tile([BPC, 1], f32, tag="first")
            second = cpool.tile([BPC, 1], f32, tag="second")
            ge2 = cpool.tile([BPC, 1], f32, tag="ge2")
            start = cpool.tile([BPC, 1], f32, tag="start")
            end = cpool.tile([BPC, 1], f32, tag="end")
            keep = cpool.tile([BPC, S], f32, tag="keep")
            keepany = cpool.tile([BPC, 1], f32, tag="keepany")
            maskneg = cpool.tile([BPC, S], f32, tag="maskneg")
            hasc = cpool.tile([BPC, 1], f32, tag="hasc")
            spos = cpool.tile([BPC, 1], f32, tag="spos")
            gidx_f = cpool.tile([BPC, 1], f32, tag="gidx_f")
            gidx_i = cpool.tile([BPC, 1], i32, tag="gidx_i")

            ts_(t_c[:], tok[:], float(AT_ID), op.is_equal)
            ts_(t_a[:], t_c[:], -1000.0, op.mult, 1000.0, op.add)
            tt_(t_a[:], t_a[:], iot[:], op.add)
            nc.vector.tensor_reduce(out=first[:], in_=t_a[:], axis=ax.X, op=op.min)
            tt_(t_b[:], iot[:], bc(first[:], [BPC, S]), op.is_gt)
            tt_(t_b[:], t_b[:], t_c[:], op.mult)
            ts_(t_b[:], t_b[:], -1000.0, op.mult, 1000.0, op.add)
            tt_(t_b[:], t_b[:], iot[:], op.add)
            nc.vector.tensor_reduce(out=second[:], in_=t_b[:], axis=ax.X, op=op.min)
            nc.vector.tensor_reduce(out=s_1[:], in_=t_c[:], axis=ax.X, op=op.add)
            ts_(ge2[:], s_1[:], 2.0, op.is_ge)
            tt_(start[:], first[:], ge2[:], op.mult)
            ts_(end[:], second[:], -float(S), op.add)
            tt_(end[:], end[:], ge2[:], op.mult)
            ts_(end[:], end[:], float(S), op.add)
            tt_(t_a[:], iot[:], bc(start[:], [BPC, S]), op.is_lt)
            tt_(t_b[:], iot[:], bc(end[:], [BPC, S]), op.is_gt)
            tt_(keep[:], t_a[:], t_b[:], op.max)
            nc.vector.tensor_reduce(out=keepany[:], in_=keep[:], axis=ax.X, op=op.max)
            ts_(maskneg[:], keep[:], -1.0, op.add, -NEG, op.mult)
            ts_(t_c[:], tok[:], float(CITSEG_ID), op.is_equal)
            ts_(t_a[:], t_c[:], -1000.0, op.mult, 1000.0, op.add)
            tt_(t_a[:], t_a[:], iot[:], op.add)
            nc.vector.tensor_reduce(out=s_1[:], in_=t_a[:], axis=ax.X, op=op.min)
            ts_(hasc[:], s_1[:], float(S - 1), op.is_le)
            ts_(spos[:], s_1[:], float(S - 1), op.min)
            ts_(gidx_f[:], biota[:], float(S), op.mult)
            tt_(gidx_f[:], gidx_f[:], spos[:], op.add)
            nc.vector.tensor_copy(out=gidx_i[:], in_=gidx_f[:])

            # ---------- weight DMAs (host pre-tiled, contiguous lines) ----------
            projw_sb = cpool.tile([P, NKT_HID, CIT], bf, tag="projw_sb")
            nc.sync.dma_start(out=projw_sb[:], in_=projw_d[:])
            encw_sb = cpool.tile([P, NKT_CIT, CIT], bf, tag="encw_sb")
            nc.sync.dma_start(out=encw_sb[:], in_=encw_d[:])
            w3sb = cpool.tile([P, NKT_H, NCLS], bf, tag="w3sb")
            nc.sync.dma_start(out=w3sb[:], in_=w3_d[:])
            projb_sb = cpool.tile([1, CIT], bf, tag="projb_sb")
            nc.sync.dma_start(out=projb_sb[:], in_=projb_d[:])
            encb_sb = cpool.tile([1, CIT], bf, tag="encb_sb")
            nc.sync.dma_start(out=encb_sb[:], in_=encb_d[:])
            b1_sb = cpool.tile([1, D2], bf, tag="b1_sb")
            nc.sync.dma_start(out=b1_sb[:], in_=b1_d[:])
            b2_sb = cpool.tile([1, D2], bf, tag="b2_sb")
            nc.sync.dma_start(out=b2_sb[:], in_=b2_d[:])
            b3_sb = cpool.tile([1, NCLS], bf, tag="b3_sb")
            nc.sync.dma_start(out=b3_sb[:], in_=b3_d[:])

            # ---------- masks -> [128 p, 4 c, 32 b] f32 (s = 4p + c) ----------
            mrearr = maskneg[:].rearrange("b (p c) -> b p c", c=4)
            mscr = cpool.tile([BPC, 4, P], f32, tag="mscr")
            for c in range(4):
                nc.vector.tensor_copy(out=mscr[:, c, :], in_=mrearr[:, :, c])
            maskcols = cpool.tile([P, 4, BPC], f32, tag="maskcols")
            for c in range(4):
                pt = pss.tile([P, BPC], f32, tag="pt_small")
                nc.tensor.transpose(
                    out=pt[:], in_=mscr[:, c, :], identity=identf[:BPC, :BPC]
                )
                nc.vector.tensor_copy(out=maskcols[:, c, :], in_=pt[:])

            hasc_row = cpool.tile([1, BPC], bf, tag="hasc_row")
            pt = pss.tile([P, BPC], f32, tag="pt_small")
            nc.tensor.transpose(out=pt[:1, :], in_=hasc[:], identity=identf[:BPC, :BPC])
            nc.vector.tensor_copy(out=hasc_row[:], in_=pt[:1, :])

            ka_row = cpool.tile([1, BPC], bf, tag="ka_row")
            pt = pss.tile([P, BPC], f32, tag="pt_small")
            nc.tensor.transpose(out=pt[:1, :], in_=keepany[:], identity=identf[:BPC, :BPC])
            nc.vector.tensor_copy(out=ka_row[:], in_=pt[:1, :])
            kab = cpool.tile([P, BPC], bf, tag="kab")
            pt = pss.tile([P, BPC], f32, tag="pt_small")
            nc.tensor.matmul(out=pt[:], lhsT=ones_row[:], rhs=ka_row[:], start=True, stop=True)
            nc.vector.tensor_copy(out=kab[:], in_=pt[:])

            # ---------- CITSEG gather + proj + enc (feature-major) ----------
            cith = cpool.tile([BPC, H], bf, tag="cith")
            hid_flat = hidden_d[:].rearrange("b s h -> (b s) h")
            nc.gpsimd.indirect_dma_start(
                out=cith[:],
                out_offset=None,
                in_=hid_flat,
                in_offset=bass.IndirectOffsetOnAxis(ap=gidx_i[:, :1], axis=0),
            )
            hasc_b = cpool.tile([BPC, 1], bf, tag="hasc_b")
            nc.vector.tensor_copy(out=hasc_b[:], in_=hasc[:])
            tt_(cith[:], cith[:], bc(hasc_b[:], [BPC, H]), op.mult)
            cithT = cpool.tile([P, NKT_HID, BPC], bf, tag="cithT")
            for t in range(NKT_HID):
                pt = psp.tile([P, 6, P], bf, tag="ptr")
                nc.tensor.transpose(
                    out=pt[:, 0, :BPC], in_=cith[:, t * P:(t + 1) * P],
                    identity=identb[:BPC, :BPC],
                )
                nc.vector.tensor_copy(out=cithT[:, t, :], in_=pt[:, 0, :BPC])

            xT = cpool.tile([P, NKT_X, BPC], bf, tag="xT")

            cpT = cpool.tile([P, NKT_CIT, BPC], bf, tag="cpT")
            for mt, (moff, msz) in enumerate(KT_CIT):
                ps = pss.tile([P, BPC], f32, tag="pt_small")
                for kt in range(NKT_HID):
                    nc.tensor.matmul(
                        out=ps[:msz, :],
                        lhsT=projw_sb[:, kt, moff:moff + msz],
                        rhs=cithT[:, kt, :],
                        start=(kt == 0), stop=False,
                    )
                nc.tensor.matmul(
                    out=ps[:msz, :], lhsT=projb_sb[:, moff:moff + msz],
                    rhs=hasc_row[:], start=False, stop=True,
                )
                nc.vector.tensor_copy(out=cpT[:msz, mt, :], in_=ps[:msz, :])

            for mt, (moff, msz) in enumerate(KT_CIT):
                ps = pss.tile([P, BPC], f32, tag="pt_small")
                for kt, (koff, ksz) in enumerate(KT_CIT):
                    nc.tensor.matmul(
                        out=ps[:msz, :],
                        lhsT=encw_sb[:ksz, kt, moff:moff + msz],
                        rhs=cpT[:ksz, kt, :],
                        start=(kt == 0), stop=False,
                    )
                nc.tensor.matmul(
                    out=ps[:msz, :], lhsT=encb_sb[:, moff:moff + msz],
                    rhs=ones_col[:], start=False, stop=True,
                )
                nc.vector.tensor_copy(out=xT[:msz, 6 + mt, :], in_=ps[:msz, :])

            # ---------- pooling: ACT 3 masked adds, GPS 1 max, DVE rest ----------
            for b in range(BPC):
                hb = hbp.tile([P, 4, H], bf, tag="hb")
                nc.sync.dma_start(
                    out=hb[:], in_=hidden_d[b].rearrange("(p c) h -> p c h", p=P)
                )
                m1 = mxp.tile([P, H], bf, tag="m1")
                m2 = mxp.tile([P, H], bf, tag="m2")
                m3 = mxp.tile([P, H], bf, tag="m3")
                acc = mxp.tile([P, H], bf, tag="acc")
                nc.scalar.activation(
                    out=m1[:], in_=hb[:, 1, :], func=act.Identity,
                    bias=maskcols[:, 1, b:b + 1], scale=1.0,
                )
                nc.scalar.activation(
                    out=m2[:], in_=hb[:, 2, :], func=act.Identity,
                    bias=maskcols[:, 2, b:b + 1], scale=1.0,
                )
                nc.scalar.activation(
                    out=m3[:], in_=hb[:, 3, :], func=act.Identity,
                    bias=maskcols[:, 3, b:b + 1], scale=1.0,
                )
                nc.vector.tensor_scalar(
                    out=acc[:], in0=hb[:, 0, :], scalar1=maskcols[:, 0, b:b + 1],
                    scalar2=None, op0=op.add,
                )
                tt_(acc[:], acc[:], m1[:], op.max)
                tt_(acc[:], acc[:], m2[:], op.max)
                tt_(acc[:], acc[:], m3[:], op.max)
                ptr = psp.tile([P, 6, P], bf, tag="ptr")
                for t in range(6):
                    nc.tensor.transpose(
                        out=ptr[:, t, :], in_=acc[:, t * P:(t + 1) * P],
                        identity=identb[:],
                    )
                nc.vector.tensor_reduce(
                    out=xT[:, 0:6, b], in_=ptr[:], axis=ax.X, op=op.max,
                )
            for t in range(6):
                tt_(xT[:, t, :], xT[:, t, :], kab[:], op.mult)

            # ---------- MLP: batch-major, 4 groups col-tiled into [128,768] ----------
            def mlp_layer(kt_list, lhsT_fn, rhs_fn, bias_sb, h_out):
                psg = psb.tile([P, 768], f32, tag="psg")
                for kt, (koff, ksz) in enumerate(kt_list):
                    for (g, goff, gw) in GRP:
                        for (c0, cw) in ((0, 512), (512, gw - 512)):
                            nc.tensor.matmul(
                                out=psg[32 * g:32 * g + 32, c0:c0 + cw],
                                lhsT=lhsT_fn(kt, ksz),
                                rhs=rhs_fn(kt, ksz, goff + c0, cw),
                                start=(kt == 0), stop=False,
                                tile_position=(0, 32 * g),
                            )
                for (g, goff, gw) in GRP:
                    for (c0, cw) in ((0, 512), (512, gw - 512)):
                        nc.tensor.matmul(
                            out=psg[32 * g:32 * g + 32, c0:c0 + cw],
                            lhsT=ones_col[:],
                            rhs=bias_sb[:, goff + c0:goff + c0 + cw],
                            start=False, stop=True,
                            tile_position=(0, 32 * g),
                        )
                nc.scalar.activation(out=h_out[:], in_=psg[:], func=act.Relu)

            def transpose_groups(h_in, hT_out):
                # h_in [128, 768]: partition b+32g holds features 768g + j
                for kt, (koff, ksz) in enumerate(KT_H):
                    g, t = kt // 6, kt % 6
                    pt = psp.tile([P, 6, P], bf, tag="ptr")
                    nc.tensor.transpose(
                        out=pt[:ksz, 0, :BPC],
                        in_=h_in[32 * g:32 * g + 32, t * P:t * P + ksz],
                        identity=identb[32 * g:32 * g + 32, 32 * g:32 * g + 32],
                        tile_position=(32 * g, 0),
                    )
                    nc.vector.tensor_copy(out=hT_out[:ksz, kt, :], in_=pt[:ksz, 0, :BPC])

            h1 = cpool.tile([P, 768], bf, tag="h1")
            psg1 = psb.tile([P, 768], f32, tag="psg")
            wt1 = None
            for kt, (koff, ksz) in enumerate(KT_X):
                if kt % 2 == 0:
                    wt1 = wp.tile([P, 2, D2], bf, tag="wt")
                    nc.sync.dma_start(out=wt1[:], in_=w1_d[kt // 2])
                for (g, goff, gw) in GRP:
                    for (c0, cw) in ((0, 512), (512, gw - 512)):
                        nc.tensor.matmul(
                            out=psg1[32 * g:32 * g + 32, c0:c0 + cw],
                            lhsT=xT[:ksz, kt, :],
                            rhs=wt1[:ksz, kt % 2, goff + c0:goff + c0 + cw],
                            start=(kt == 0), stop=False,
                            tile_position=(0, 32 * g),
                        )
            for (g, goff, gw) in GRP:
                for (c0, cw) in ((0, 512), (512, gw - 512)):
                    nc.tensor.matmul(
                        out=psg1[32 * g:32 * g + 32, c0:c0 + cw],
                        lhsT=ones_col[:],
                        rhs=b1_sb[:, goff + c0:goff + c0 + cw],
                        start=False, stop=True,
                        tile_position=(0, 32 * g),
                    )
            nc.scalar.activation(out=h1[:], in_=psg1[:], func=act.Relu)
            h1T = cpool.tile([P, NKT_H, BPC], bf, tag="h1T")
            transpose_groups(h1, h1T)

            h2 = cpool.tile([P, 768], bf, tag="h2")
            psg = psb.tile([P, 768], f32, tag="psg")
            wt = None
            for kt, (koff, ksz) in enumerate(KT_H):
                if kt % 2 == 0:
                    wt = wp.tile([P, 2, D2], bf, tag="wt")
                    nc.sync.dma_start(out=wt[:], in_=w2_d[kt // 2])
                for (g, goff, gw) in GRP:
                    for (c0, cw) in ((0, 512), (512, gw - 512)):
                        nc.tensor.matmul(
                            out=psg[32 * g:32 * g + 32, c0:c0 + cw],
                            lhsT=h1T[:ksz, kt, :],
                            rhs=wt[:ksz, kt % 2, goff + c0:goff + c0 + cw],
                            start=(kt == 0), stop=False,
                            tile_position=(0, 32 * g),
                        )
            for (g, goff, gw) in GRP:
                for (c0, cw) in ((0, 512), (512, gw - 512)):
                    nc.tensor.matmul(
                        out=psg[32 * g:32 * g + 32, c0:c0 + cw],
                        lhsT=ones_col[:],
                        rhs=b2_sb[:, goff + c0:goff + c0 + cw],
                        start=False, stop=True,
                        tile_position=(0, 32 * g),
                    )
            nc.scalar.activation(out=h2[:], in_=psg[:], func=act.Relu)

            h2T = cpool.tile([P, NKT_H, BPC], bf, tag="h2T")
            transpose_groups(h2, h2T)

            pso = pss.tile([P, BPC], f32, tag="pt_small")
            for kt, (koff, ksz) in enumerate(KT_H):
                nc.tensor.matmul(
                    out=pso[:BPC, :NCLS],
                    lhsT=h2T[:ksz, kt, :],
                    rhs=w3sb[:ksz, kt, :],
                    start=(kt == 0), stop=False,
                )
            nc.tensor.matmul(
                out=pso[:BPC, :NCLS], lhsT=ones_col[:],
                rhs=b3_sb[:], start=False, stop=True,
            )
            out_sb = cpool.tile([BPC, NCLS], f32, tag="out_sb")
            nc.vector.tensor_copy(out=out_sb[:], in_=pso[:BPC, :NCLS])
            nc.sync.dma_start(out=out_d[:], in_=out_sb[:])

    nc.compile()
    return nc


def _get_nc():
    if "nc" not in _CACHED:
        _CACHED["nc"] = _build_bass()
    return _CACHED["nc"]


def _ktile_pack(w, n_kt, bfl):
    """[K, M] f32 -> [128, n_kt, M] bf16 with zero row padding."""
    K, M = w.shape
    wp_ = np.zeros((n_kt * 128, M), dtype=np.float32)
    wp_[:K] = w
    return np.ascontiguousarray(
        wp_.reshape(n_kt, 128, M).transpose(1, 0, 2).astype(bfl)
    )


def _chunk_pack(w, n_ch, bfl):
    """[K, M] f32 -> [n_ch, 128, 2, M] bf16, 2 k-tiles per chunk, zero pad."""
    K, M = w.shape
    wp_ = np.zeros((n_ch * 256, M), dtype=np.float32)
    wp_[:K] = w
    return np.ascontiguousarray(
        wp_.reshape(n_ch, 2, 128, M).transpose(0, 2, 1, 3).astype(bfl)
    )


def kernel(**inputs) -> np.ndarray:
    from concourse.bass_utils import run_bass_kernel_spmd
    import ml_dtypes

    bfl = ml_dtypes.bfloat16
    nc = _get_nc()

    tokens = np.asarray(inputs["tokens"]).astype(np.int32)
    hidden = np.ascontiguousarray(
        np.asarray(inputs["hidden_states"], dtype=np.float32).astype(bfl)
    )
    w2h = _chunk_pack(np.asarray(inputs["w2"], dtype=np.float32), NW2, bfl)

    def vcast(name, shape):
        return np.ascontiguousarray(
            np.asarray(inputs[name], dtype=np.float32).astype(bfl).reshape(shape)
        )

    shared = {
        "proj_w": _ktile_pack(np.asarray(inputs["proj_w"], np.float32), NKT_HID, bfl),
        "proj_b": vcast("proj_b", (1, CIT)),
        "enc_w": _ktile_pack(np.asarray(inputs["enc_w"], np.float32), NKT_CIT, bfl),
        "enc_b": vcast("enc_b", (1, CIT)),
        "w1": _chunk_pack(np.asarray(inputs["w1"], np.float32), 6, bfl),
        "b1": vcast("b1", (1, D2)),
        "w2": w2h,
        "b2": vcast("b2", (1, D2)),
        "w3": _ktile_pack(np.asarray(inputs["w3"], np.float32), NKT_H, bfl),
        "b3": vcast("b3", (1, NCLS)),
    }
    in_maps = []
    for i in range(NCORES):
        sl = slice(i * BPC, (i + 1) * BPC)
        m = dict(shared)
        m["tokens"] = np.ascontiguousarray(tokens[sl])
        m["hidden"] = np.ascontiguousarray(hidden[sl])
        in_maps.append(m)

    res = run_bass_kernel_spmd(
        nc, in_maps, core_ids=list(range(NCORES)), trace=bool(_CACHED.get("trace")),
        tmpdir=_CACHED.get("tmpdir"),
    )
    _CACHED["last_res"] = res
    out = np.concatenate([res.results[i]["out"] for i in range(NCORES)], axis=0)
    return out.astype(np.float32)


if __name__ == "__main__":
    rng = np.random.default_rng(0)
    ins = {
        "tokens": rng.integers(0, 100, (B, S)).astype(np.int64),
        "hidden_states": rng.standard_normal((B, S, H)).astype(np.float32),
        "proj_w": (rng.standard_normal((H, CIT)) / np.sqrt(H)).astype(np.float32),
        "proj_b": (rng.standard_normal(CIT) * 0.02).astype(np.float32),
        "enc_w": (rng.standard_normal((CIT, CIT)) / np.sqrt(CIT)).astype(np.float32),
        "enc_b": (rng.standard_normal(CIT) * 0.02).astype(np.float32),
        "w1": (rng.standard_normal((D1, D2)) / np.sqrt(D1)).astype(np.float32),
        "b1": (rng.standard_normal(D2) * 0.02).astype(np.float32),
        "w2": (rng.standard_normal((D2, D2)) / np.sqrt(D2)).astype(np.float32),
        "b2": (rng.standard_normal(D2) * 0.02).astype(np.float32),
        "w3": (rng.standard_normal((D2, NCLS)) / np.sqrt(D2)).astype(np.float32),
        "b3": (rng.standard_normal(NCLS) * 0.02).astype(np.float32),
    }
    got = kernel(**ins)
    print("kernel out", got.shape, got.dtype, got[:2])
